# revision 7
# baseline (speedup 1.0000x reference)
"""Trainium2 Bass kernel for DNAS PreBasicBlock (mixed-quantization residual block).

Math:
  out = residual + mixed_qconv2(bn_relu2(mixed_qconv1(bn_relu1(x))))

Key optimizations (see git-less history in comments):
  * relu+clip fold; candidate folding by activation bits (3 convs per layer);
    A_4 = round(A_8/17), A_2 = round(A_4/5) derived on the fly per superchunk.
  * layer-1 weights hi/lo bf16 (near-fp32); layer-2 single fp16 pass (x256).
  * warmup collective at t=0 absorbs the CC engine's one-time algo/library
    setup (~50us) so the BN-stats AllReduce runs at warm latency.
  * x DMA split 16 ways (one queue each); weight DMAs queued right behind
    (2 pieces per candidate), single DMA pass per layer: raws stay resident
    in raw9 and tanh/quant chain runs in place.
  * BN batch stats: DVE computes sums, ACT computes sum-of-squares
    (halves the serial ACT time on the allreduce trigger path).
  * full W1 prep emitted before conv; first superchunk still starts with
    group 2 only so the PE ramps as early as possible.
  * layer-2 stats allreduce split: everything except img3's tail rows goes
    out while superchunk 8 computes; only the tail rides the critical path.
  * first image quantized in two steps (rows 0..29 first) so the first
    superchunk's derive never waits on a full-image pass.
  * conv as 9 shift-matmuls per pass accumulating in PSUM; matmuls reuse
    loaded PE weights across the 3 chunks of a superchunk (ldweights skip).
"""
import sys

sys.path.insert(0, "/opt/trn_rl_repo")

import numpy as np

import concourse.bass as bass
import concourse.tile as tile
from concourse import bacc, bass_utils, mybir

dt = mybir.dt
Alu = mybir.AluOpType
Act = mybir.ActivationFunctionType

N_CORES = 8
B, C, H, W = 32, 128, 56, 56
BS = B // N_CORES          # batch shard per core
HP, WP = H + 2, W + 2      # padded image: 1 row/col of zeros on each side
IMG = HP * WP              # 3364
APIX = BS * IMG            # 13456
BASE = WP + 1              # first valid flat offset within an image: 59
VSTART = BASE
VEND = (BS - 1) * IMG + H * WP + W + 1  # one past last valid: 13397
CHUNK = 512
NCHUNK = -(-(VEND - VSTART) // CHUNK)  # 27
SCCH = 3                   # chunks per superchunk
SLEN = SCCH * CHUNK        # 1536
NSC = -(-NCHUNK // SCCH)   # 9
STG = 1664                 # derive-staging width (halo + alignment slack)
NPIX_IMG = H * W           # 3136
IMG_SPAN = H * WP          # 3248: rows 1..56 as 56 x 58 view
NTOT = float(B * H * W)    # BN divisor 100352
MAGIC = 12582912.0         # 1.5*2^23: fp32 round-to-int via add/sub
MAGICB = 192.0             # 1.5*2^7: bf16 round-to-int via f32->bf16 convert
MAGICH = 1536.0            # 1.5*2^10: fp16 round-to-int via f32->fp16 convert
EPS = 1e-5
W2SCALE = 256.0            # layer-2 fp16 weight scale (off subnormals)
LDW_REUSE = True           # skip PE weight reload on repeat-lhsT matmuls

BITS = [2, 4, 8]
NW = [2 ** BITS[k // 3] - 1 for k in range(9)]   # weight levels per candidate
NA = [2 ** BITS[k % 3] - 1 for k in range(9)]    # activation levels per candidate
KORDER = [2, 5, 8, 1, 4, 7, 0, 3, 6]             # group-major, ba=8 group first
GROUP_KS = {2: [2, 5, 8], 1: [1, 4, 7], 0: [0, 3, 6]}
TAPS = [(ky - 1) * WP + (kx - 1) for ky in range(3) for kx in range(3)]

# which superchunks become available after which image is quantized:
SC_NEEDS_IMG = []
for _s in range(NSC):
    _hi = min(VSTART + 1536 * (_s + 1) + 59, APIX)
    SC_NEEDS_IMG.append(min((_hi - 1) // IMG, BS - 1))

# image i's last valid pixel lives in superchunk:
IMG_LAST_SC = {}
for _i in range(BS):
    _last = _i * IMG + H * WP + W
    _s = min((_last - VSTART) // SLEN, NSC - 1)
    IMG_LAST_SC[_s] = _i

# rows of img0 needed before superchunk 0 can derive+run: pixels [0, STG)
ROWS_SC0 = -(-(STG - BASE) // WP) + 1  # 29

_CACHE = {}


def _chunks_of_sc(s):
    out = []
    for c in range(SCCH):
        ci = s * SCCH + c
        if ci >= NCHUNK:
            break
        gs = VSTART + ci * CHUNK
        ln = min(CHUNK, VEND - gs)
        out.append((c * CHUNK, gs, ln))
    return out


def _build():
    nc = bacc.Bacc("TRN2", target_bir_lowering=False, debug=False,
                   num_devices=N_CORES)

    x_in = nc.dram_tensor("x", [BS, C, H, W], dt.float32, kind="ExternalInput")
    w1_in = nc.dram_tensor("conv1_w", [9, C, C, 3, 3], dt.float32, kind="ExternalInput")
    w2_in = nc.dram_tensor("conv2_w", [9, C, C, 3, 3], dt.float32, kind="ExternalInput")
    g1_in = nc.dram_tensor("gamma1", [C], dt.float32, kind="ExternalInput")
    b1_in = nc.dram_tensor("beta1", [C], dt.float32, kind="ExternalInput")
    g2_in = nc.dram_tensor("gamma2", [C], dt.float32, kind="ExternalInput")
    b2_in = nc.dram_tensor("beta2", [C], dt.float32, kind="ExternalInput")
    p1_in = nc.dram_tensor("p1", [9], dt.float32, kind="ExternalInput")
    p2_in = nc.dram_tensor("p2", [9], dt.float32, kind="ExternalInput")
    gn1_in = nc.dram_tensor("gn1", [9], dt.float32, kind="ExternalInput")
    gn2_in = nc.dram_tensor("gn2", [9], dt.float32, kind="ExternalInput")
    tau_in = nc.dram_tensor("tau", [1], dt.float32, kind="ExternalInput")
    consts_in = nc.dram_tensor("consts", [1, 27], dt.float32, kind="ExternalInput")
    out_dram = nc.dram_tensor("out", [BS, C, H, W], dt.float32, kind="ExternalOutput")

    from concourse.masks import make_identity

    with tile.TileContext(nc) as tc:
        with tc.tile_pool(name="main", bufs=1) as sb, \
             tc.tile_pool(name="ps", bufs=1, space="PSUM") as ps, \
             tc.tile_pool(name="dram", bufs=1, space="DRAM") as dram:

            # ---------- warmup collective: absorb CC one-time setup ----------
            warm_in = dram.tile([1, 1], dt.float32, name="warm_in")
            warm_out = dram.tile([1, 1], dt.float32, addr_space="Shared",
                                 name="warm_out")
            nc.gpsimd.collective_compute(
                "AllReduce", Alu.add,
                replica_groups=[list(range(N_CORES))],
                ins=[warm_in.opt()], outs=[warm_out.opt()])
            warm_sb = sb.tile([1, 1], dt.float32, name="warm_sb")
            nc.sync.dma_start(warm_sb[:], warm_out[:])

            # ---------- static tiles / input DMAs ----------
            ident = sb.tile([128, 128], dt.float32)
            make_identity(nc, ident[:])

            A8 = sb.tile([C, APIX], dt.bfloat16, tag="A8")
            nc.gpsimd.memset(A8[:], 0.0)  # zero borders once; writes stay interior

            x_sb = sb.tile([C, BS * NPIX_IMG], dt.float32, tag="big", name="x_sb")
            x_src = x_in.ap().rearrange("b c h w -> c b (h w)")
            QTR = NPIX_IMG // 4
            for i in range(BS):
                for qq in range(4):  # 16 pieces -> all DMA queues
                    lo_ = (i * 4 + qq) * QTR
                    nc.sync.dma_start(
                        x_sb[:, lo_:lo_ + QTR],
                        x_src[:, i, qq * QTR:(qq + 1) * QTR])

            # weight raws: single DMA pass, resident; queued behind x
            raw9 = sb.tile([C, 9 * 1152], dt.float32, tag="raw9", name="raw9")
            wsrc1 = w1_in.ap().rearrange("k o i a b -> k o (i a b)")
            for k in KORDER:
                for hh in range(2):
                    nc.sync.dma_start(
                        raw9[:, k * 1152 + hh * 576:k * 1152 + (hh + 1) * 576],
                        wsrc1[k][:, hh * 576:(hh + 1) * 576])

            def row(name, t, n):
                r = sb.tile([1, n], dt.float32, name=name)
                nc.sync.dma_start(r[:], t.ap()[None, :])
                return r

            p1r = row("p1r", p1_in, 9)
            gn1r = row("gn1r", gn1_in, 9)
            p2r = row("p2r", p2_in, 9)
            gn2r = row("gn2r", gn2_in, 9)
            taur = row("taur", tau_in, 1)
            constsr = sb.tile([1, 27], dt.float32)
            nc.sync.dma_start(constsr[:], consts_in.ap())

            def col128(name, t):
                r = sb.tile([C, 1], dt.float32, name=name)
                nc.sync.dma_start(r[:], t.ap()[:, None])
                return r

            gam1, bet1 = col128("gam1", g1_in), col128("bet1", b1_in)
            gam2, bet2 = col128("gam2", g2_in), col128("bet2", b2_in)

            rtau = sb.tile([1, 1], dt.float32)
            nc.vector.reciprocal(rtau[:], taur[:])

            # ---------- per-layer softmax -> alpha/gamma strip -> broadcast ----------
            def softmax_strip(pr, gnr, tag):
                u = sb.tile([1, 9], dt.float32, name=f"u_{tag}")
                nc.vector.tensor_tensor(u[:], pr[:], gnr[:], Alu.add)
                nc.vector.tensor_scalar(u[:], u[:], rtau[:, 0:1], None, Alu.mult)
                mx = sb.tile([1, 1], dt.float32, name=f"mx_{tag}")
                nc.vector.tensor_reduce(mx[:], u[:], axis=mybir.AxisListType.X,
                                        op=Alu.max)
                nmx = sb.tile([1, 1], dt.float32, name=f"nmx_{tag}")
                nc.vector.tensor_scalar(nmx[:], mx[:], -1.0, None, Alu.mult)
                e = sb.tile([1, 9], dt.float32, name=f"e_{tag}")
                nc.scalar.activation(e[:], u[:], Act.Exp, bias=nmx[:, 0:1], scale=1.0)
                ssum = sb.tile([1, 1], dt.float32, name=f"ss_{tag}")
                nc.vector.tensor_reduce(ssum[:], e[:], axis=mybir.AxisListType.X,
                                        op=Alu.add)
                rsum = sb.tile([1, 1], dt.float32, name=f"rs_{tag}")
                nc.vector.reciprocal(rsum[:], ssum[:])
                wrow = sb.tile([1, 9], dt.float32, name=f"w_{tag}")
                nc.vector.tensor_scalar(wrow[:], e[:], rsum[:, 0:1], None, Alu.mult)
                strip = sb.tile([1, 12], dt.float32, name=f"strip_{tag}")
                nc.vector.tensor_tensor(strip[:, 0:9], wrow[:], constsr[:, 0:9],
                                        Alu.mult)
                pe1 = sb.tile([1, 9], dt.float32, name=f"pe1_{tag}")
                nc.vector.tensor_tensor(pe1[:], wrow[:], constsr[:, 9:18], Alu.mult)
                pe13 = pe1[:].rearrange("p (i g) -> p i g", g=3)
                for g in range(3):
                    nc.vector.tensor_reduce(strip[:, 9 + g:10 + g], pe13[:, :, g],
                                            axis=mybir.AxisListType.X, op=Alu.add,
                                            negate=True)
                bcast = sb.tile([C, 12], dt.float32, name=f"bcast_{tag}")
                nc.gpsimd.partition_broadcast(bcast[:], strip[:])
                return bcast

            # ---------- BN batch stats: DVE sums, ACT sum-of-squares ----------
            def img_stats(src3d, stats_cols, col, tag, src2d=None):
                a = src3d.shape[1]
                if src2d is not None:  # contiguous image: one-shot row reduce
                    nc.vector.tensor_reduce(stats_cols[:, col:col + 1], src2d,
                                            axis=mybir.AxisListType.X, op=Alu.add)
                else:  # strided view: reduce innermost, then the row of rows
                    rs = sb.tile([C, H], dt.float32, tag="rsum", bufs=2,
                                 name=f"rs_{tag}_{col}")
                    nc.vector.tensor_reduce(rs[:, 0:a], src3d,
                                            axis=mybir.AxisListType.X, op=Alu.add)
                    nc.vector.tensor_reduce(stats_cols[:, col:col + 1], rs[:, 0:a],
                                            axis=mybir.AxisListType.X, op=Alu.add)
                n = src3d.shape[1] * src3d.shape[2]
                scr2 = sb.tile([C, NPIX_IMG], dt.float32, tag="scr", bufs=2,
                               name=f"scq_{tag}_{col}")
                scr23 = scr2[:, 0:n].rearrange("p (a b) -> p a b",
                                               b=src3d.shape[2])
                nc.scalar.activation(scr23, src3d, Act.Square, bias=0.0, scale=1.0,
                                     accum_out=stats_cols[:, 5 + col:6 + col])

            def allreduce_cols(loc, tag):
                cin = dram.tile([C, 2], dt.float32, name=f"ccin_{tag}")
                cout = dram.tile([C, 2], dt.float32, addr_space="Shared",
                                 name=f"ccout_{tag}")
                nc.sync.dma_start(cin[:], loc[:])
                nc.gpsimd.collective_compute(
                    "AllReduce", Alu.add,
                    replica_groups=[list(range(N_CORES))],
                    ins=[cin.opt()], outs=[cout.opt()])
                glob = sb.tile([C, 2], dt.float32, name=f"glob_{tag}")
                nc.sync.dma_start(glob[:], cout[:])
                return glob

            def reduce_stats(stats_cols, tag, lo=0, hi=5):
                loc = sb.tile([C, 2], dt.float32, name=f"loc_{tag}")
                sc3 = stats_cols[:].rearrange("p (s i) -> p s i", s=2)
                nc.vector.tensor_reduce(loc[:], sc3[:, :, lo:hi],
                                        axis=mybir.AxisListType.X, op=Alu.add)
                return loc

            def bn_scalars(glob, gam, bet, tag):
                def t1(name):
                    return sb.tile([C, 1], dt.float32, name=f"{name}_{tag}")
                mean, e2, msq, var, ve = (t1("mean"), t1("e2"), t1("msq"),
                                          t1("var"), t1("ve"))
                nc.vector.tensor_scalar(mean[:], glob[:, 0:1], 1.0 / NTOT, None,
                                        Alu.mult)
                nc.vector.tensor_scalar(e2[:], glob[:, 1:2], 1.0 / NTOT, None,
                                        Alu.mult)
                nc.vector.tensor_tensor(msq[:], mean[:], mean[:], Alu.mult)
                nc.vector.tensor_tensor(var[:], e2[:], msq[:], Alu.subtract)
                nc.vector.tensor_scalar(ve[:], var[:], EPS, None, Alu.add)
                sq, y = t1("sq"), t1("y0")
                nc.scalar.activation(sq[:], ve[:], Act.Sqrt, bias=0.0, scale=1.0)
                nc.vector.reciprocal(y[:], sq[:])
                for it in range(2):  # Newton: y <- y*(1.5 - 0.5*ve*y^2)
                    tt1, tt2, tt3, yn = (t1(f"n{it}a"), t1(f"n{it}b"),
                                         t1(f"n{it}c"), t1(f"y{it + 1}"))
                    nc.vector.tensor_tensor(tt1[:], y[:], y[:], Alu.mult)
                    nc.vector.tensor_tensor(tt2[:], tt1[:], ve[:], Alu.mult)
                    nc.vector.tensor_scalar(tt3[:], tt2[:], -0.5, 1.5, Alu.mult,
                                            Alu.add)
                    nc.vector.tensor_tensor(yn[:], y[:], tt3[:], Alu.mult)
                    y = yn
                sbn, bt, sq_, bq_ = (t1("sbn"), t1("bt"), t1("sclq"), t1("biasq"))
                nc.vector.tensor_tensor(sbn[:], gam[:], y[:], Alu.mult)
                nc.vector.tensor_tensor(bt[:], mean[:], sbn[:], Alu.mult)
                # u = 255*(s*x + b): sq = 255*s ; bq = 255*(beta - mean*s)
                nc.vector.tensor_scalar(bq_[:], bt[:], -255.0, bet255(bet, tag),
                                        Alu.mult, Alu.add)
                nc.vector.tensor_scalar(sq_[:], sbn[:], 255.0, None, Alu.mult)
                return sq_, bq_

            _bet255 = {}

            def bet255(bet, tag):
                if tag not in _bet255:
                    b = sb.tile([C, 1], dt.float32, name=f"bet255_{tag}")
                    nc.vector.tensor_scalar(b[:], bet[:], 255.0, None, Alu.mult)
                    _bet255[tag] = b
                return _bet255[tag][:, 0:1]

            # ---------- quantize rows [r0, r1) of one image into the 8-bit grid ----------
            def quantize_img(src3d_full, A8t, sq_, bq_, i, tag, r0=0, r1=H):
                # u = relu(255*(s*x+b)); round+clamp: (min(u,255)+M)-M
                src3d = src3d_full[:, r0:r1]
                n = (r1 - r0) * W
                u = sb.tile([C, NPIX_IMG], dt.float32, tag="scr", bufs=2,
                            name=f"qu_{tag}_{i}_{r0}")
                u3 = u[:, 0:n].rearrange("p (a b) -> p a b", a=r1 - r0)
                nc.scalar.activation(u3, src3d, Act.Relu, bias=bq_[:, 0:1],
                                     scale=sq_[:, 0:1])
                nc.vector.tensor_scalar(u[:, 0:n], u[:, 0:n], 255.0, MAGIC,
                                        Alu.min, Alu.add)
                dst = A8t[:, i * IMG + BASE + r0 * WP:
                          i * IMG + BASE + r1 * WP]
                dst3 = dst.rearrange("p (a b) -> p a b", b=WP)[:, :, 0:W]
                nc.vector.tensor_scalar(dst3, u3, MAGIC, None, Alu.subtract)

            # ---------- weight preparation ----------
            def prep_amax(tag):
                # partition-axis max via PE transpose + DVE reduce + tiny DMA
                amax = sb.tile([C, 9], dt.float32, name=f"amax_{tag}")
                for k in KORDER:
                    nc.vector.tensor_reduce(amax[:, k:k + 1],
                                            raw9[:, k * 1152:(k + 1) * 1152],
                                            axis=mybir.AxisListType.X, op=Alu.max,
                                            apply_absolute_value=True)
                tp9 = ps.tile([9, 128], dt.float32, tag="tps", bufs=2,
                              name=f"tp9_{tag}")
                nc.tensor.transpose(tp9[:], amax[:], ident[:])
                mx9 = sb.tile([9, 1], dt.float32, name=f"mx9_{tag}")
                nc.vector.tensor_reduce(mx9[:], tp9[:], axis=mybir.AxisListType.X,
                                        op=Alu.max)
                mrow = sb.tile([1, 9], dt.float32, name=f"mrow_{tag}")
                for k in range(9):
                    nc.sync.dma_start(mrow[0:1, k:k + 1], mx9[k:k + 1, 0:1])
                tam = sb.tile([1, 9], dt.float32, name=f"tam_{tag}")
                nc.scalar.activation(tam[:], mrow[:], Act.Tanh, bias=0.0, scale=1.0)
                a2 = sb.tile([1, 9], dt.float32, name=f"a2_{tag}")
                nc.vector.tensor_scalar(a2[:], tam[:], 2.0, None, Alu.mult)
                r2r = sb.tile([1, 9], dt.float32, name=f"r2r_{tag}")
                nc.vector.reciprocal(r2r[:], a2[:])
                r2 = sb.tile([C, 9], dt.float32, name=f"r2_{tag}")
                nc.gpsimd.partition_broadcast(r2[:], r2r[:])
                return r2

            def prep_tanh(tag):
                # tanh in place over the resident raws (after amax extraction)
                for k in KORDER:
                    v = raw9[:, k * 1152:(k + 1) * 1152]
                    nc.scalar.activation(v, v, Act.Tanh, bias=0.0, scale=1.0)

            def prep_chain(r2, bcast, tag, g, wacc_bufs=2):
                """accumulate one ba-group's quantized candidates -> wacc."""
                wacc = None
                for pos, k in enumerate(GROUP_KS[g]):
                    th = raw9[:, k * 1152:(k + 1) * 1152]
                    # wn = th/(2amax)+0.5 ; u2 = wn*nw + M (rounds) ; m = u2-M
                    # (+0.5*nw must NOT fold into M: M+0.5nw isn't fp32-exact)
                    nc.vector.tensor_scalar(th, th, r2[:, k:k + 1], 0.5,
                                            Alu.mult, Alu.add)
                    nc.vector.tensor_scalar(th, th, float(NW[k]), MAGIC,
                                            Alu.mult, Alu.add)
                    nc.vector.tensor_scalar(th, th, MAGIC, None,
                                            Alu.subtract)
                    if pos == 0:
                        wacc = sb.tile([C, 1152], dt.float32, tag=f"wacc_{tag}",
                                       bufs=wacc_bufs, name=f"wacc_{tag}_{g}_{pos}")
                        nc.vector.tensor_scalar(wacc[:], th, bcast[:, k:k + 1],
                                                bcast[:, 9 + g:10 + g],
                                                Alu.mult, Alu.add)
                    else:
                        nxt = sb.tile([C, 1152], dt.float32, tag=f"wacc_{tag}",
                                      bufs=wacc_bufs, name=f"wacc_{tag}_{g}_{pos}")
                        nc.vector.scalar_tensor_tensor(nxt[:], th,
                                                       bcast[:, k:k + 1], wacc[:],
                                                       Alu.mult, Alu.add)
                        wacc = nxt
                return wacc

            def prep_transpose(wacc, tag, g, Wt, fp16):
                w3 = wacc[:].rearrange("p (i t) -> p i t", t=9)
                for t in range(9):
                    tp = ps.tile([128, 128], dt.float32, tag="tps", bufs=2,
                                 name=f"tp_{tag}_{g}_{t}")
                    nc.tensor.transpose(tp[:], w3[:, :, t], ident[:])
                    if fp16:
                        nc.scalar.activation(Wt[g][0][:, t, :], tp[:], Act.Copy,
                                             bias=0.0, scale=W2SCALE)
                    else:
                        nc.scalar.activation(Wt[g][0][:, t, :], tp[:], Act.Copy,
                                             bias=0.0, scale=1.0)
                        nc.vector.tensor_tensor(Wt[g][1][:, t, :], tp[:],
                                                Wt[g][0][:, t, :], Alu.subtract)

            def alloc_W(tag, fp16):
                wdtype = dt.float16 if fp16 else dt.bfloat16
                nh = 1 if fp16 else 2
                return [[sb.tile([C, 9, C], wdtype, name=f"W_{tag}_{g}_{h}")
                         for h in range(nh)] for g in range(3)]

            # ---------- conv pieces ----------
            def derive_sc(src8, s, tag, fp16):
                adtype = dt.float16 if fp16 else dt.bfloat16
                magic = MAGICH if fp16 else MAGICB
                start = VSTART + s * SLEN
                lo = min(max((start - 64) & ~1, 0), APIX - STG)
                a4s = sb.tile([C, STG], adtype, tag="a4s", bufs=3,
                              name=f"a4_{tag}_{s}")
                nc.vector.tensor_scalar(a4s[:], src8[:, lo:lo + STG], 1.0 / 17.0,
                                        magic, Alu.mult, Alu.add)
                nc.vector.tensor_scalar(a4s[:], a4s[:], magic, None, Alu.subtract)
                a2s = sb.tile([C, STG], adtype, tag="a2s", bufs=3,
                              name=f"a2_{tag}_{s}")
                nc.vector.tensor_scalar(a2s[:], a4s[:], 1.0 / 5.0, magic,
                                        Alu.mult, Alu.add)
                nc.vector.tensor_scalar(a2s[:], a2s[:], magic, None, Alu.subtract)
                return a4s, a2s, lo

            def conv_sc(Wt, src8, a4s, a2s, lo, cdst, s, tag, fp16, groups=None):
                """emit conv passes for superchunk s; groups=None -> all."""
                chunks = _chunks_of_sc(s)
                start = VSTART + s * SLEN
                if fp16:
                    all_passes = [(2, 0), (1, 0), (0, 0)]
                else:
                    all_passes = [(2, 0), (2, 1), (1, 0), (1, 1), (0, 0), (0, 1)]
                passes = [(pi, gh) for pi, gh in enumerate(all_passes)
                          if groups is None or gh[0] in groups]
                pt = _sc_psum(tag, s)
                for pi, (g, hh) in passes:
                    for t in range(9):
                        off = TAPS[t]
                        for ci, (pcol, gs, ln) in enumerate(chunks):
                            if g == 2:
                                rhs = src8[:, gs + off:gs + off + ln]
                            elif g == 1:
                                rhs = a4s[:, gs + off - lo:gs + off - lo + ln]
                            else:
                                rhs = a2s[:, gs + off - lo:gs + off - lo + ln]
                            nc.tensor.matmul(
                                pt[:, pcol:pcol + ln], Wt[g][hh][:, t, :], rhs,
                                start=(pi == 0 and t == 0),
                                stop=(pi == len(all_passes) - 1 and t == 8))
                if groups is None or all_passes[-1][0] in groups:
                    sc_end = min(start + SLEN, VEND)
                    nc.scalar.activation(cdst[:, start:sc_end],
                                         pt[:, 0:sc_end - start], Act.Copy,
                                         bias=0.0,
                                         scale=(1.0 / W2SCALE if fp16 else 1.0))

            _psums = {}

            def _sc_psum(tag, s):
                key = (tag, s)
                if key not in _psums:
                    _psums[key] = ps.tile([128, SLEN], dt.float32, tag="cps",
                                          bufs=2, name=f"ps_{tag}_{s}")
                return _psums[key]

            # ================= LAYER 1 =================
            stats1 = sb.tile([C, 10], dt.float32)
            nc.vector.memset(stats1[:], 0.0)
            x3 = x_sb[:].rearrange("p (b a w) -> p b a w", b=BS, a=H)
            for i in range(BS):
                img_stats(x3[:, i], stats1, i, "s1",
                          src2d=x_sb[:, i * NPIX_IMG:(i + 1) * NPIX_IMG])

            loc1 = reduce_stats(stats1, "c1")
            glob1 = allreduce_cols(loc1, "c1")

            bc1 = softmax_strip(p1r, gn1r, "l1")
            bc2 = softmax_strip(p2r, gn2r, "l2")

            r2_1 = prep_amax("w1")
            prep_tanh("w1")

            W1 = alloc_W("w1", False)
            c1 = sb.tile([C, APIX], dt.float32, tag="big", name="c1buf")
            stats2 = sb.tile([C, 10], dt.float32)
            nc.vector.memset(stats2[:], 0.0)

            # group-2 chain first (gates the first conv pass), interleaved
            # with the BN->quantize critical path on the in-order DVE queue
            wacc = prep_chain(r2_1, bc1, "w1", 2)
            nsq1, nbq1 = bn_scalars(glob1, gam1, bet1, "bn1")
            quantize_img(x3[:, 0], A8, nsq1, nbq1, 0, "q1", 0, ROWS_SC0)
            a4s0, a2s0, lo0 = derive_sc(A8[:], 0, "cv1", False)
            prep_transpose(wacc, "w1", 2, W1, False)
            quantize_img(x3[:, 0], A8, nsq1, nbq1, 0, "q1", ROWS_SC0, H)
            quantize_img(x3[:, 1], A8, nsq1, nbq1, 1, "q1")
            # NOTE: all of x must be consumed (quantized) before conv1's first
            # PSUM copy writes c1 -- they share one SBUF slot and the slot
            # handover is tile-granular.

            def c1_img3d(i, r0=0, r1=H):
                off = i * IMG + BASE + r0 * WP
                v = c1[:, off:off + (r1 - r0) * WP]
                return v.rearrange("p (a b) -> p a b", b=WP)[:, :, 0:W]

            conv_sc(W1, A8[:], a4s0, a2s0, lo0, c1, 0, "cv1", False, groups=[2])
            wacc = prep_chain(r2_1, bc1, "w1", 1)
            prep_transpose(wacc, "w1", 1, W1, False)
            a4s1, a2s1, lo1 = derive_sc(A8[:], 1, "cv1", False)
            conv_sc(W1, A8[:], a4s0, a2s0, lo0, c1, 0, "cv1", False, groups=[1])
            wacc = prep_chain(r2_1, bc1, "w1", 0)
            prep_transpose(wacc, "w1", 0, W1, False)
            quantize_img(x3[:, 2], A8, nsq1, nbq1, 2, "q1")
            quantize_img(x3[:, 3], A8, nsq1, nbq1, 3, "q1")
            conv_sc(W1, A8[:], a4s0, a2s0, lo0, c1, 0, "cv1", False, groups=[0])
            conv_sc(W1, A8[:], a4s1, a2s1, lo1, c1, 1, "cv1", False)

            def cv1_after_sc(s):
                if s == 7:  # partial img3 stats (rows 0..37 available)
                    img_stats(c1_img3d(3, 0, 37), stats2, 3, "s2")
                    loc2a = reduce_stats(stats2, "c2a", 0, 4)
                    return allreduce_cols(loc2a, "c2a")
                if s in IMG_LAST_SC:
                    i = IMG_LAST_SC[s]
                    if i != 3:
                        img_stats(c1_img3d(i), stats2, i, "s2")
                return None

            cv1_after_sc(0), cv1_after_sc(1)
            glob2a = None
            w2src = w2_in.ap().rearrange("k o i a b -> k o (i a b)")
            for s in range(2, NSC):
                a4s, a2s, lo = derive_sc(A8[:], s, "cv1", False)
                conv_sc(W1, A8[:], a4s, a2s, lo, c1, s, "cv1", False)
                ret = cv1_after_sc(s)
                if ret is not None:
                    glob2a = ret
                if s == 2:
                    # layer-2 raws reuse the raw9 slot once layer-1 chains
                    # are consumed; spread across conv1 superchunks
                    for k in KORDER:
                        for hh in range(2):
                            nc.sync.dma_start(
                                raw9[:, k * 1152 + hh * 576:
                                     k * 1152 + (hh + 1) * 576],
                                w2src[k][:, hh * 576:(hh + 1) * 576])
                elif s == 3:
                    r2_2 = prep_amax("w2")
                    prep_tanh("w2")
                    W2 = alloc_W("w2", True)
                    wacc_g = prep_chain(r2_2, bc2, "w2", 2)
                    prep_transpose(wacc_g, "w2", 2, W2, True)
                elif s == 4:
                    wacc_g = prep_chain(r2_2, bc2, "w2", 1)
                    prep_transpose(wacc_g, "w2", 1, W2, True)
                elif s == 5:
                    wacc_g = prep_chain(r2_2, bc2, "w2", 0)
                    prep_transpose(wacc_g, "w2", 0, W2, True)

            # ================= LAYER 2 =================
            # tail stats (img3 rows 37..56) -> tiny allreduce; everything else
            # went out during superchunk 8
            img_stats(c1_img3d(3, 37, H), stats2, 4, "s2t")
            loc2b = reduce_stats(stats2, "c2b", 4, 5)
            glob2b = allreduce_cols(loc2b, "c2b")
            glob2 = sb.tile([C, 2], dt.float32, name="glob2")
            nc.vector.tensor_tensor(glob2[:], glob2a[:], glob2b[:], Alu.add)
            nsq2, nbq2 = bn_scalars(glob2, gam2, bet2, "bn2")

            # fp16 grid shares the A8 slot: zero borders are bit-identical
            A8h = sb.tile([C, APIX], dt.float16, tag="A8", name="A8h")

            out_v = out_dram.ap().rearrange("b c h w -> c b (h w)")

            def residual_out(i, r0=0, r1=H):
                n = (r1 - r0) * W
                xr = sb.tile([C, NPIX_IMG], dt.float32, tag="scr", bufs=2,
                             name=f"xr_{i}_{r0}")
                nc.sync.dma_start(xr[:, 0:n], x_src[:, i, r0 * W:r1 * W])
                xr3 = xr[:, 0:n].rearrange("p (a b) -> p a b", a=r1 - r0)
                nc.vector.tensor_tensor(xr3, xr3, c1_img3d(i, r0, r1), Alu.add)
                nc.sync.dma_start(out_v[:, i, r0 * W:r1 * W], xr[:, 0:n])

            quantize_img(c1_img3d(0), A8h, nsq2, nbq2, 0, "q2", 0, ROWS_SC0)
            _q2_done = {}
            for s in range(0, NSC):
                need = SC_NEEDS_IMG[s]
                for i in range(BS):
                    if need >= i and not _q2_done.get(i):
                        if i == 0:
                            quantize_img(c1_img3d(0), A8h, nsq2, nbq2, 0, "q2",
                                         ROWS_SC0, H)
                        else:
                            quantize_img(c1_img3d(i), A8h, nsq2, nbq2, i, "q2")
                        _q2_done[i] = True
                a4s, a2s, lo = derive_sc(A8h[:], s, "cv2", True)
                conv_sc(W2, A8h[:], a4s, a2s, lo, c1, s, "cv2", True)
                if s == 7:
                    residual_out(3, 0, 37)
                if s in IMG_LAST_SC:
                    i = IMG_LAST_SC[s]
                    if i == 3:
                        residual_out(3, 37, H)
                    else:
                        residual_out(i)

    nc.compile()

    if LDW_REUSE:
        # drop PE weight reloads that repeat the previous load verbatim (the
        # array already holds these weights); only sync-free loads come out
        for blk in nc.main_func.blocks:
            keep, prev = [], None
            for inst in blk.instructions:
                if isinstance(inst, mybir.InstLdweights):
                    w = inst.ins[0]
                    key = (w.memref, w.offset, str(w.ap))
                    si = inst.sync_info
                    clean = si is None or (len(si.on_wait) == 0
                                           and len(si.on_update) == 0)
                    if key == prev and clean:
                        continue
                    prev = key
                elif isinstance(inst, mybir.InstMatmult) and inst.is_transpose:
                    prev = None
                keep.append(inst)
            blk.instructions = keep
    return nc


def _consts():
    c = np.zeros((1, 27), np.float32)
    for k in range(9):
        c[0, k] = 2.0 / (NW[k] * NA[k])
        c[0, 9 + k] = 1.0 / NA[k]
        c[0, 18 + k] = float(NW[k])
    return c


def _in_maps(inputs):
    x = np.ascontiguousarray(inputs["x"], dtype=np.float32)
    shared = {
        "conv1_w": np.ascontiguousarray(inputs["conv1_w"], dtype=np.float32),
        "conv2_w": np.ascontiguousarray(inputs["conv2_w"], dtype=np.float32),
        "gamma1": np.ascontiguousarray(inputs["gamma1"], dtype=np.float32),
        "beta1": np.ascontiguousarray(inputs["beta1"], dtype=np.float32),
        "gamma2": np.ascontiguousarray(inputs["gamma2"], dtype=np.float32),
        "beta2": np.ascontiguousarray(inputs["beta2"], dtype=np.float32),
        "p1": np.ascontiguousarray(inputs["p1"], dtype=np.float32),
        "p2": np.ascontiguousarray(inputs["p2"], dtype=np.float32),
        "gn1": np.ascontiguousarray(inputs["gn1"], dtype=np.float32),
        "gn2": np.ascontiguousarray(inputs["gn2"], dtype=np.float32),
        "tau": np.asarray(inputs["tau"], dtype=np.float32).reshape(1),
        "consts": _consts(),
    }
    return [dict(shared, x=x[c * BS:(c + 1) * BS]) for c in range(N_CORES)]


def _get_nc():
    if "nc" not in _CACHE:
        _CACHE["nc"] = _build()
    return _CACHE["nc"]


def _run(in_maps, trace=False):
    nc = _get_nc()
    return bass_utils.run_bass_kernel_spmd(
        nc, in_maps, core_ids=list(range(N_CORES)), trace=trace)


def kernel(**inputs) -> np.ndarray:
    res = _run(_in_maps(inputs))
    return np.concatenate([res.results[c]["out"] for c in range(N_CORES)], axis=0)


# revision 23
# speedup vs baseline: 1.3514x; 1.3514x over previous
"""Trainium2 Bass kernel for DNAS PreBasicBlock (mixed-quantization residual block).

Math:
  out = residual + mixed_qconv2(bn_relu2(mixed_qconv1(bn_relu1(x))))

Key optimizations (see git-less history in comments):
  * relu+clip fold; candidate folding by activation bits (3 convs per layer);
    A_4 = round(A_8/17), A_2 = round(A_4/5) derived on the fly per superchunk.
  * layer-1 weights hi/lo bf16 (near-fp32); layer-2 single fp16 pass (x256).
  * warmup collective at t=0 absorbs the CC engine's one-time algo/library
    setup (~50us) so the BN-stats AllReduce runs at warm latency.
  * x DMA split 16 ways (one queue each); weight DMAs queued right behind
    (2 pieces per candidate), single DMA pass per layer: raws stay resident
    in raw9 and tanh/quant chain runs in place.
  * BN batch stats: DVE computes sums, ACT computes sum-of-squares
    (halves the serial ACT time on the allreduce trigger path).
  * full W1 prep emitted before conv; first superchunk still starts with
    group 2 only so the PE ramps as early as possible.
  * layer-2 stats allreduce split: everything except img3's tail rows goes
    out while superchunk 8 computes; only the tail rides the critical path.
  * first image quantized in two steps (rows 0..29 first) so the first
    superchunk's derive never waits on a full-image pass.
  * conv as 9 shift-matmuls per pass accumulating in PSUM; matmuls reuse
    loaded PE weights across the 3 chunks of a superchunk (ldweights skip).
"""
import sys

sys.path.insert(0, "/opt/trn_rl_repo")

import numpy as np

import concourse.bass as bass
import concourse.tile as tile
from concourse import bacc, bass_utils, mybir

dt = mybir.dt
Alu = mybir.AluOpType
Act = mybir.ActivationFunctionType

N_CORES = 8
B, C, H, W = 32, 128, 56, 56
BS = B // N_CORES          # batch shard per core
HP, WP = H + 2, W + 2      # padded image: 1 row/col of zeros on each side
IMG = HP * WP              # 3364
APIX = BS * IMG            # 13456
BASE = WP + 1              # first valid flat offset within an image: 59
VSTART = BASE
VEND = (BS - 1) * IMG + H * WP + W + 1  # one past last valid: 13397
CHUNK = 512
NCHUNK = -(-(VEND - VSTART) // CHUNK)  # 27
SCCH = 3                   # chunks per superchunk
SLEN = SCCH * CHUNK        # 1536
NSC = -(-NCHUNK // SCCH)   # 9
STG = 1664                 # derive-staging width (halo + alignment slack)
NPIX_IMG = H * W           # 3136
IMG_SPAN = H * WP          # 3248: rows 1..56 as 56 x 58 view
NTOT = float(B * H * W)    # BN divisor 100352
MAGIC = 12582912.0         # 1.5*2^23: fp32 round-to-int via add/sub
MAGICH = 1536.0            # 1.5*2^10: fp16 round-to-int via f32->fp16 convert
EPS = 1e-5
WSCALE = 256.0             # fp16 weight scale (off subnormals)
LDW_REUSE = True           # skip PE weight reload on repeat-lhsT matmuls

BITS = [2, 4, 8]
NW = [2 ** BITS[k // 3] - 1 for k in range(9)]   # weight levels per candidate
NA = [2 ** BITS[k % 3] - 1 for k in range(9)]    # activation levels per candidate
KORDER = [2, 5, 8, 1, 4, 7, 0, 3, 6]             # group-major, ba=8 group first
GROUP_KS = {2: [2, 5, 8], 1: [1, 4, 7], 0: [0, 3, 6]}
TAPS = [(ky - 1) * WP + (kx - 1) for ky in range(3) for kx in range(3)]

# which superchunks become available after which image is quantized:
SC_NEEDS_IMG = []
for _s in range(NSC):
    _hi = min(VSTART + 1536 * (_s + 1) + 59, APIX)
    SC_NEEDS_IMG.append(min((_hi - 1) // IMG, BS - 1))

# image i's last valid pixel lives in superchunk:
IMG_LAST_SC = {}
for _i in range(BS):
    _last = _i * IMG + H * WP + W
    _s = min((_last - VSTART) // SLEN, NSC - 1)
    IMG_LAST_SC[_s] = _i

# rows of img0 needed before superchunk 0 can derive+run: pixels [0, STG)
ROWS_SC0 = -(-(STG - BASE) // WP) + 1  # 29

_CACHE = {}


def _chunks_of_sc(s):
    out = []
    for c in range(SCCH):
        ci = s * SCCH + c
        if ci >= NCHUNK:
            break
        gs = VSTART + ci * CHUNK
        ln = min(CHUNK, VEND - gs)
        out.append((c * CHUNK, gs, ln))
    return out


def _build():
    nc = bacc.Bacc("TRN2", target_bir_lowering=False, debug=False,
                   num_devices=N_CORES)

    x_in = nc.dram_tensor("x", [BS, C, H, W], dt.float32, kind="ExternalInput")
    w1_in = nc.dram_tensor("conv1_w", [9, C, C, 3, 3], dt.float32, kind="ExternalInput")
    w2_in = nc.dram_tensor("conv2_w", [9, C, C, 3, 3], dt.float32, kind="ExternalInput")
    g1_in = nc.dram_tensor("gamma1", [C], dt.float32, kind="ExternalInput")
    b1_in = nc.dram_tensor("beta1", [C], dt.float32, kind="ExternalInput")
    g2_in = nc.dram_tensor("gamma2", [C], dt.float32, kind="ExternalInput")
    b2_in = nc.dram_tensor("beta2", [C], dt.float32, kind="ExternalInput")
    p1_in = nc.dram_tensor("p1", [9], dt.float32, kind="ExternalInput")
    p2_in = nc.dram_tensor("p2", [9], dt.float32, kind="ExternalInput")
    gn1_in = nc.dram_tensor("gn1", [9], dt.float32, kind="ExternalInput")
    gn2_in = nc.dram_tensor("gn2", [9], dt.float32, kind="ExternalInput")
    tau_in = nc.dram_tensor("tau", [1], dt.float32, kind="ExternalInput")
    consts_in = nc.dram_tensor("consts", [1, 27], dt.float32, kind="ExternalInput")
    out_dram = nc.dram_tensor("out", [BS, C, H, W], dt.float32, kind="ExternalOutput")

    from concourse.masks import make_identity

    with tile.TileContext(nc) as tc:
        with tc.tile_pool(name="main", bufs=1) as sb, \
             tc.tile_pool(name="ps", bufs=1, space="PSUM") as ps, \
             tc.tile_pool(name="dram", bufs=1, space="DRAM") as dram:

            # ---------- static tiles / input DMAs ----------
            ident = sb.tile([128, 128], dt.float32)
            make_identity(nc, ident[:])

            # A8 memset is emitted later (after the stats-allreduce trigger) so
            # the gpsimd queue reaches that trigger without an 11us stall
            A8 = sb.tile([C, APIX], dt.float16, tag="A8")

            x_sb = sb.tile([C, BS * NPIX_IMG], dt.float32, tag="big", name="x_sb")
            x_src = x_in.ap().rearrange("b c h w -> c b (h w)")
            QTR = NPIX_IMG // 4
            for i in range(BS):
                for qq in range(4):  # 16 pieces -> all DMA queues
                    lo_ = (i * 4 + qq) * QTR
                    nc.sync.dma_start(
                        x_sb[:, lo_:lo_ + QTR],
                        x_src[:, i, qq * QTR:(qq + 1) * QTR])

            # weight raws: single DMA pass, resident; queued behind x
            raw9 = sb.tile([C, 9 * 1152], dt.float32, tag="raw9", name="raw9")
            wsrc1 = w1_in.ap().rearrange("k o i a b -> k o (i a b)")
            for k in KORDER:
                for hh in range(2):
                    nc.sync.dma_start(
                        raw9[:, k * 1152 + hh * 576:k * 1152 + (hh + 1) * 576],
                        wsrc1[k][:, hh * 576:(hh + 1) * 576])

            def row(name, t, n):
                r = sb.tile([1, n], dt.float32, name=name)
                nc.sync.dma_start(r[:], t.ap()[None, :])
                return r

            p1r = row("p1r", p1_in, 9)
            gn1r = row("gn1r", gn1_in, 9)
            p2r = row("p2r", p2_in, 9)
            gn2r = row("gn2r", gn2_in, 9)
            taur = row("taur", tau_in, 1)
            constsr = sb.tile([1, 27], dt.float32)
            nc.sync.dma_start(constsr[:], consts_in.ap())

            def col128(name, t):
                r = sb.tile([C, 1], dt.float32, name=name)
                nc.sync.dma_start(r[:], t.ap()[:, None])
                return r

            gam1, bet1 = col128("gam1", g1_in), col128("bet1", b1_in)
            gam2, bet2 = col128("gam2", g2_in), col128("bet2", b2_in)

            rtau = sb.tile([1, 1], dt.float32)
            nc.vector.reciprocal(rtau[:], taur[:])

            # ---------- per-layer softmax -> alpha/gamma strip -> broadcast ----------
            def softmax_strip(pr, gnr, tag):
                u = sb.tile([1, 9], dt.float32, name=f"u_{tag}")
                nc.vector.tensor_tensor(u[:], pr[:], gnr[:], Alu.add)
                nc.vector.tensor_scalar(u[:], u[:], rtau[:, 0:1], None, Alu.mult)
                mx = sb.tile([1, 1], dt.float32, name=f"mx_{tag}")
                nc.vector.tensor_reduce(mx[:], u[:], axis=mybir.AxisListType.X,
                                        op=Alu.max)
                nmx = sb.tile([1, 1], dt.float32, name=f"nmx_{tag}")
                nc.vector.tensor_scalar(nmx[:], mx[:], -1.0, None, Alu.mult)
                e = sb.tile([1, 9], dt.float32, name=f"e_{tag}")
                nc.scalar.activation(e[:], u[:], Act.Exp, bias=nmx[:, 0:1], scale=1.0)
                ssum = sb.tile([1, 1], dt.float32, name=f"ss_{tag}")
                nc.vector.tensor_reduce(ssum[:], e[:], axis=mybir.AxisListType.X,
                                        op=Alu.add)
                rsum = sb.tile([1, 1], dt.float32, name=f"rs_{tag}")
                nc.vector.reciprocal(rsum[:], ssum[:])
                wrow = sb.tile([1, 9], dt.float32, name=f"w_{tag}")
                nc.vector.tensor_scalar(wrow[:], e[:], rsum[:, 0:1], None, Alu.mult)
                strip = sb.tile([1, 12], dt.float32, name=f"strip_{tag}")
                nc.vector.tensor_tensor(strip[:, 0:9], wrow[:], constsr[:, 0:9],
                                        Alu.mult)
                pe1 = sb.tile([1, 9], dt.float32, name=f"pe1_{tag}")
                nc.vector.tensor_tensor(pe1[:], wrow[:], constsr[:, 9:18], Alu.mult)
                pe13 = pe1[:].rearrange("p (i g) -> p i g", g=3)
                for g in range(3):
                    nc.vector.tensor_reduce(strip[:, 9 + g:10 + g], pe13[:, :, g],
                                            axis=mybir.AxisListType.X, op=Alu.add,
                                            negate=True)
                bcast = sb.tile([C, 12], dt.float32, name=f"bcast_{tag}")
                nc.gpsimd.partition_broadcast(bcast[:], strip[:])
                return bcast

            # ---------- BN batch stats: DVE sums, ACT sum-of-squares ----------
            def img_stats(src3d, stats_cols, col, tag, src2d=None):
                a = src3d.shape[1]
                if src2d is not None:  # contiguous image: one-shot row reduce
                    nc.vector.tensor_reduce(stats_cols[:, col:col + 1], src2d,
                                            axis=mybir.AxisListType.X, op=Alu.add)
                else:  # strided view: reduce innermost, then the row of rows
                    rs = sb.tile([C, H], dt.float32, tag="rsum", bufs=2,
                                 name=f"rs_{tag}_{col}")
                    nc.vector.tensor_reduce(rs[:, 0:a], src3d,
                                            axis=mybir.AxisListType.X, op=Alu.add)
                    nc.vector.tensor_reduce(stats_cols[:, col:col + 1], rs[:, 0:a],
                                            axis=mybir.AxisListType.X, op=Alu.add)
                n = src3d.shape[1] * src3d.shape[2]
                scr2 = sb.tile([C, NPIX_IMG], dt.float32, tag="scr", bufs=2,
                               name=f"scq_{tag}_{col}")
                scr23 = scr2[:, 0:n].rearrange("p (a b) -> p a b",
                                               b=src3d.shape[2])
                nc.scalar.activation(scr23, src3d, Act.Square, bias=0.0, scale=1.0,
                                     accum_out=stats_cols[:, 5 + col:6 + col])

            def allreduce_cols(loc, tag):
                cin = dram.tile([C, 2], dt.float32, name=f"ccin_{tag}")
                cout = dram.tile([C, 2], dt.float32, addr_space="Shared",
                                 name=f"ccout_{tag}")
                nc.sync.dma_start(cin[:], loc[:])
                nc.gpsimd.collective_compute(
                    "AllReduce", Alu.add,
                    replica_groups=[list(range(N_CORES))],
                    ins=[cin.opt()], outs=[cout.opt()])
                glob = sb.tile([C, 2], dt.float32, name=f"glob_{tag}")
                nc.sync.dma_start(glob[:], cout[:])
                return glob

            def reduce_stats(stats_cols, tag, lo=0, hi=5):
                loc = sb.tile([C, 2], dt.float32, name=f"loc_{tag}")
                sc3 = stats_cols[:].rearrange("p (s i) -> p s i", s=2)
                nc.vector.tensor_reduce(loc[:], sc3[:, :, lo:hi],
                                        axis=mybir.AxisListType.X, op=Alu.add)
                return loc

            def bn_scalars(glob, gam, bet, tag):
                def t1(name):
                    return sb.tile([C, 1], dt.float32, name=f"{name}_{tag}")
                mean, e2, msq, var, ve = (t1("mean"), t1("e2"), t1("msq"),
                                          t1("var"), t1("ve"))
                nc.vector.tensor_scalar(mean[:], glob[:, 0:1], 1.0 / NTOT, None,
                                        Alu.mult)
                nc.vector.tensor_scalar(e2[:], glob[:, 1:2], 1.0 / NTOT, None,
                                        Alu.mult)
                nc.vector.tensor_tensor(msq[:], mean[:], mean[:], Alu.mult)
                nc.vector.tensor_tensor(var[:], e2[:], msq[:], Alu.subtract)
                nc.vector.tensor_scalar(ve[:], var[:], EPS, None, Alu.add)
                sq, y = t1("sq"), t1("y0")
                nc.scalar.activation(sq[:], ve[:], Act.Sqrt, bias=0.0, scale=1.0)
                nc.vector.reciprocal(y[:], sq[:])
                for it in range(2):  # Newton: y <- y*(1.5 - 0.5*ve*y^2)
                    tt1, tt2, tt3, yn = (t1(f"n{it}a"), t1(f"n{it}b"),
                                         t1(f"n{it}c"), t1(f"y{it + 1}"))
                    nc.vector.tensor_tensor(tt1[:], y[:], y[:], Alu.mult)
                    nc.vector.tensor_tensor(tt2[:], tt1[:], ve[:], Alu.mult)
                    nc.vector.tensor_scalar(tt3[:], tt2[:], -0.5, 1.5, Alu.mult,
                                            Alu.add)
                    nc.vector.tensor_tensor(yn[:], y[:], tt3[:], Alu.mult)
                    y = yn
                sbn, bt, sq_, bq_ = (t1("sbn"), t1("bt"), t1("sclq"), t1("biasq"))
                nc.vector.tensor_tensor(sbn[:], gam[:], y[:], Alu.mult)
                nc.vector.tensor_tensor(bt[:], mean[:], sbn[:], Alu.mult)
                # u = 255*(s*x + b): sq = 255*s ; bq = 255*(beta - mean*s)
                nc.vector.tensor_scalar(bq_[:], bt[:], -255.0, bet255(bet, tag),
                                        Alu.mult, Alu.add)
                nc.vector.tensor_scalar(sq_[:], sbn[:], 255.0, None, Alu.mult)
                return sq_, bq_

            _bet255 = {}

            def bet255(bet, tag):
                if tag not in _bet255:
                    b = sb.tile([C, 1], dt.float32, name=f"bet255_{tag}")
                    nc.vector.tensor_scalar(b[:], bet[:], 255.0, None, Alu.mult)
                    _bet255[tag] = b
                return _bet255[tag][:, 0:1]

            # ---------- quantize rows [r0, r1) of one image into the 8-bit grid ----------
            def quantize_img(src3d_full, A8t, sq_, bq_, i, tag, r0=0, r1=H):
                # u = relu(255*(s*x+b)); round+clamp: (min(u,255)+M)-M
                src3d = src3d_full[:, r0:r1]
                n = (r1 - r0) * W
                u = sb.tile([C, NPIX_IMG], dt.float32, tag="scr", bufs=2,
                            name=f"qu_{tag}_{i}_{r0}")
                u3 = u[:, 0:n].rearrange("p (a b) -> p a b", a=r1 - r0)
                nc.scalar.activation(u3, src3d, Act.Relu, bias=bq_[:, 0:1],
                                     scale=sq_[:, 0:1])
                nc.vector.tensor_scalar(u[:, 0:n], u[:, 0:n], 255.0, MAGIC,
                                        Alu.min, Alu.add)
                dst = A8t[:, i * IMG + BASE + r0 * WP:
                          i * IMG + BASE + r1 * WP]
                dst3 = dst.rearrange("p (a b) -> p a b", b=WP)[:, :, 0:W]
                nc.vector.tensor_scalar(dst3, u3, MAGIC, None, Alu.subtract)

            # ---------- weight preparation ----------
            def prep_amax(tag):
                # partition-axis max via PE transpose + DVE reduce + tiny DMA
                amax = sb.tile([C, 9], dt.float32, name=f"amax_{tag}")
                for k in KORDER:
                    nc.vector.tensor_reduce(amax[:, k:k + 1],
                                            raw9[:, k * 1152:(k + 1) * 1152],
                                            axis=mybir.AxisListType.X, op=Alu.max,
                                            apply_absolute_value=True)
                tp9 = ps.tile([9, 128], dt.float32, tag="tps", bufs=2,
                              name=f"tp9_{tag}")
                nc.tensor.transpose(tp9[:], amax[:], ident[:])
                mx9 = sb.tile([9, 1], dt.float32, name=f"mx9_{tag}")
                nc.vector.tensor_reduce(mx9[:], tp9[:], axis=mybir.AxisListType.X,
                                        op=Alu.max)
                mrow = sb.tile([1, 9], dt.float32, name=f"mrow_{tag}")
                for k in range(9):
                    nc.sync.dma_start(mrow[0:1, k:k + 1], mx9[k:k + 1, 0:1])
                tam = sb.tile([1, 9], dt.float32, name=f"tam_{tag}")
                nc.scalar.activation(tam[:], mrow[:], Act.Tanh, bias=0.0, scale=1.0)
                a2 = sb.tile([1, 9], dt.float32, name=f"a2_{tag}")
                nc.vector.tensor_scalar(a2[:], tam[:], 2.0, None, Alu.mult)
                r2r = sb.tile([1, 9], dt.float32, name=f"r2r_{tag}")
                nc.vector.reciprocal(r2r[:], a2[:])
                r2 = sb.tile([C, 9], dt.float32, name=f"r2_{tag}")
                nc.gpsimd.partition_broadcast(r2[:], r2r[:])
                return r2

            def prep_tanh(tag):
                # tanh in place over the resident raws (after amax extraction)
                for k in KORDER:
                    v = raw9[:, k * 1152:(k + 1) * 1152]
                    nc.scalar.activation(v, v, Act.Tanh, bias=0.0, scale=1.0)

            def prep_chain(r2, bcast, tag, g):
                """accumulate one ba-group's quantized candidates -> wacc."""
                wacc = None
                for pos, k in enumerate(GROUP_KS[g]):
                    th = raw9[:, k * 1152:(k + 1) * 1152]
                    # wn = th/(2amax)+0.5 ; u2 = wn*nw + M (rounds) ; m = u2-M
                    # (+0.5*nw must NOT fold into M: M+0.5nw isn't fp32-exact)
                    nc.vector.tensor_scalar(th, th, r2[:, k:k + 1], 0.5,
                                            Alu.mult, Alu.add)
                    nc.vector.tensor_scalar(th, th, float(NW[k]), MAGIC,
                                            Alu.mult, Alu.add)
                    nc.vector.tensor_scalar(th, th, MAGIC, None,
                                            Alu.subtract)
                    if pos == 0:
                        wacc = sb.tile([C, 1152], dt.float32, tag="wacc",
                                       bufs=2, name=f"wacc_{tag}_{g}_{pos}")
                        nc.vector.tensor_scalar(wacc[:], th, bcast[:, k:k + 1],
                                                bcast[:, 9 + g:10 + g],
                                                Alu.mult, Alu.add)
                    else:
                        nxt = sb.tile([C, 1152], dt.float32, tag="wacc",
                                      bufs=2, name=f"wacc_{tag}_{g}_{pos}")
                        nc.vector.scalar_tensor_tensor(nxt[:], th,
                                                       bcast[:, k:k + 1], wacc[:],
                                                       Alu.mult, Alu.add)
                        wacc = nxt
                return wacc

            def prep_transpose(wacc, tag, g, Wt):
                w3 = wacc[:].rearrange("p (i t) -> p i t", t=9)
                for t in range(9):
                    tp = ps.tile([128, 128], dt.float32, tag="tps", bufs=2,
                                 name=f"tp_{tag}_{g}_{t}")
                    nc.tensor.transpose(tp[:], w3[:, :, t], ident[:])
                    nc.scalar.activation(Wt[g][:, t, :], tp[:], Act.Copy,
                                         bias=0.0, scale=WSCALE)

            def alloc_W(tag):
                # fp16 x256: single-pass weights at 16-bit PE streaming rate
                return [sb.tile([C, 9, C], dt.float16, name=f"W_{tag}_{g}")
                        for g in range(3)]

            # ---------- conv pieces ----------
            def derive_sc(src8, s, tag):
                start = VSTART + s * SLEN
                lo = min(max((start - 64) & ~1, 0), APIX - STG)
                a4s = sb.tile([C, STG], dt.float16, tag="a4s", bufs=3,
                              name=f"a4_{tag}_{s}")
                nc.vector.tensor_scalar(a4s[:], src8[:, lo:lo + STG], 1.0 / 17.0,
                                        MAGICH, Alu.mult, Alu.add)
                nc.vector.tensor_scalar(a4s[:], a4s[:], MAGICH, None, Alu.subtract)
                a2s = sb.tile([C, STG], dt.float16, tag="a2s", bufs=3,
                              name=f"a2_{tag}_{s}")
                nc.vector.tensor_scalar(a2s[:], a4s[:], 1.0 / 5.0, MAGICH,
                                        Alu.mult, Alu.add)
                nc.vector.tensor_scalar(a2s[:], a2s[:], MAGICH, None, Alu.subtract)
                return a4s, a2s, lo

            def conv_sc(Wt, src8, a4s, a2s, lo, cdst, s, tag, groups=None):
                """emit conv passes for superchunk s; groups=None -> all."""
                chunks = _chunks_of_sc(s)
                start = VSTART + s * SLEN
                all_passes = [2, 1, 0]
                passes = [(pi, g) for pi, g in enumerate(all_passes)
                          if groups is None or g in groups]
                pt = _sc_psum(tag, s)
                for pi, g in passes:
                    for t in range(9):
                        off = TAPS[t]
                        for ci, (pcol, gs, ln) in enumerate(chunks):
                            if g == 2:
                                rhs = src8[:, gs + off:gs + off + ln]
                            elif g == 1:
                                rhs = a4s[:, gs + off - lo:gs + off - lo + ln]
                            else:
                                rhs = a2s[:, gs + off - lo:gs + off - lo + ln]
                            nc.tensor.matmul(
                                pt[:, pcol:pcol + ln], Wt[g][:, t, :], rhs,
                                start=(pi == 0 and t == 0),
                                stop=(pi == len(all_passes) - 1 and t == 8))
                if groups is None or all_passes[-1] in groups:
                    sc_end = min(start + SLEN, VEND)
                    nc.scalar.activation(cdst[:, start:sc_end],
                                         pt[:, 0:sc_end - start], Act.Copy,
                                         bias=0.0, scale=1.0 / WSCALE)

            _psums = {}

            def _sc_psum(tag, s):
                key = (tag, s)
                if key not in _psums:
                    _psums[key] = ps.tile([128, SLEN], dt.float32, tag="cps",
                                          bufs=2, name=f"ps_{tag}_{s}")
                return _psums[key]

            # ================= LAYER 1 =================
            stats1 = sb.tile([C, 10], dt.float32)
            nc.vector.memset(stats1[:], 0.0)
            x3 = x_sb[:].rearrange("p (b a w) -> p b a w", b=BS, a=H)
            for i in range(BS):
                img_stats(x3[:, i], stats1, i, "s1",
                          src2d=x_sb[:, i * NPIX_IMG:(i + 1) * NPIX_IMG])

            loc1 = reduce_stats(stats1, "c1")
            glob1 = allreduce_cols(loc1, "c1")

            nc.gpsimd.memset(A8[:], 0.0)  # zero borders once; writes stay interior

            bc1 = softmax_strip(p1r, gn1r, "l1")
            bc2 = softmax_strip(p2r, gn2r, "l2")

            r2_1 = prep_amax("w1")
            prep_tanh("w1")

            W1 = alloc_W("w1")
            c1 = sb.tile([C, APIX], dt.float32, tag="big", name="c1buf")
            stats2 = sb.tile([C, 10], dt.float32)
            nc.vector.memset(stats2[:], 0.0)

            # group-2 chain first (gates the first conv pass), interleaved
            # with the BN->quantize critical path on the in-order DVE queue
            wacc = prep_chain(r2_1, bc1, "w1", 2)
            nsq1, nbq1 = bn_scalars(glob1, gam1, bet1, "bn1")
            quantize_img(x3[:, 0], A8, nsq1, nbq1, 0, "q1", 0, ROWS_SC0)
            a4s0, a2s0, lo0 = derive_sc(A8[:], 0, "cv1")
            prep_transpose(wacc, "w1", 2, W1)
            quantize_img(x3[:, 0], A8, nsq1, nbq1, 0, "q1", ROWS_SC0, H)
            quantize_img(x3[:, 1], A8, nsq1, nbq1, 1, "q1")
            # NOTE: all of x must be consumed (quantized) before conv1's first
            # PSUM copy writes c1 -- they share one SBUF slot and the slot
            # handover is tile-granular.

            def c1_img3d(i, r0=0, r1=H):
                off = i * IMG + BASE + r0 * WP
                v = c1[:, off:off + (r1 - r0) * WP]
                return v.rearrange("p (a b) -> p a b", b=WP)[:, :, 0:W]

            conv_sc(W1, A8[:], a4s0, a2s0, lo0, c1, 0, "cv1", groups=[2])
            wacc = prep_chain(r2_1, bc1, "w1", 1)
            prep_transpose(wacc, "w1", 1, W1)
            a4s1, a2s1, lo1 = derive_sc(A8[:], 1, "cv1")
            conv_sc(W1, A8[:], a4s0, a2s0, lo0, c1, 0, "cv1", groups=[1])
            wacc = prep_chain(r2_1, bc1, "w1", 0)
            prep_transpose(wacc, "w1", 0, W1)
            quantize_img(x3[:, 2], A8, nsq1, nbq1, 2, "q1")
            quantize_img(x3[:, 3], A8, nsq1, nbq1, 3, "q1")
            conv_sc(W1, A8[:], a4s0, a2s0, lo0, c1, 0, "cv1", groups=[0])
            conv_sc(W1, A8[:], a4s1, a2s1, lo1, c1, 1, "cv1")

            def cv1_after_sc(s):
                if s == 7:  # partial img3 stats (rows 0..37 available)
                    img_stats(c1_img3d(3, 0, 37), stats2, 3, "s2")
                    loc2a = reduce_stats(stats2, "c2a", 0, 4)
                    return allreduce_cols(loc2a, "c2a")
                if s in IMG_LAST_SC:
                    i = IMG_LAST_SC[s]
                    if i != 3:
                        img_stats(c1_img3d(i), stats2, i, "s2")
                return None

            cv1_after_sc(0), cv1_after_sc(1)
            glob2a = None
            w2src = w2_in.ap().rearrange("k o i a b -> k o (i a b)")
            for s in range(2, NSC):
                a4s, a2s, lo = derive_sc(A8[:], s, "cv1")
                conv_sc(W1, A8[:], a4s, a2s, lo, c1, s, "cv1")
                ret = cv1_after_sc(s)
                if ret is not None:
                    glob2a = ret
                if s == 2:
                    # layer-2 raws reuse the raw9 slot once layer-1 chains
                    # are consumed; spread across conv1 superchunks
                    for k in KORDER:
                        for hh in range(2):
                            nc.sync.dma_start(
                                raw9[:, k * 1152 + hh * 576:
                                     k * 1152 + (hh + 1) * 576],
                                w2src[k][:, hh * 576:(hh + 1) * 576])
                elif s == 3:
                    r2_2 = prep_amax("w2")
                    prep_tanh("w2")
                    W2 = alloc_W("w2")
                    wacc_g = prep_chain(r2_2, bc2, "w2", 2)
                    prep_transpose(wacc_g, "w2", 2, W2)
                elif s == 4:
                    wacc_g = prep_chain(r2_2, bc2, "w2", 1)
                    prep_transpose(wacc_g, "w2", 1, W2)
                elif s == 5:
                    wacc_g = prep_chain(r2_2, bc2, "w2", 0)
                    prep_transpose(wacc_g, "w2", 0, W2)

            # ================= LAYER 2 =================
            # tail stats (img3 rows 37..56) -> tiny allreduce; everything else
            # went out during superchunk 8
            img_stats(c1_img3d(3, 37, H), stats2, 4, "s2t")
            loc2b = reduce_stats(stats2, "c2b", 4, 5)
            glob2b = allreduce_cols(loc2b, "c2b")
            glob2 = sb.tile([C, 2], dt.float32, name="glob2")
            nc.vector.tensor_tensor(glob2[:], glob2a[:], glob2b[:], Alu.add)
            nsq2, nbq2 = bn_scalars(glob2, gam2, bet2, "bn2")

            # layer 2 reuses the A8 grid tile (conv1 is done with it)
            out_v = out_dram.ap().rearrange("b c h w -> c b (h w)")

            def residual_out(i, r0=0, r1=H):
                n = (r1 - r0) * W
                xr = sb.tile([C, NPIX_IMG], dt.float32, tag="scr", bufs=2,
                             name=f"xr_{i}_{r0}")
                nc.sync.dma_start(xr[:, 0:n], x_src[:, i, r0 * W:r1 * W])
                xr3 = xr[:, 0:n].rearrange("p (a b) -> p a b", a=r1 - r0)
                nc.vector.tensor_tensor(xr3, xr3, c1_img3d(i, r0, r1), Alu.add)
                nc.sync.dma_start(out_v[:, i, r0 * W:r1 * W], xr[:, 0:n])

            quantize_img(c1_img3d(0), A8, nsq2, nbq2, 0, "q2", 0, ROWS_SC0)
            _q2_done = {}
            for s in range(0, NSC):
                need = SC_NEEDS_IMG[s]
                for i in range(BS):
                    if need >= i and not _q2_done.get(i):
                        if i == 0:
                            quantize_img(c1_img3d(0), A8, nsq2, nbq2, 0, "q2",
                                         ROWS_SC0, H)
                        else:
                            quantize_img(c1_img3d(i), A8, nsq2, nbq2, i, "q2")
                        _q2_done[i] = True
                a4s, a2s, lo = derive_sc(A8[:], s, "cv2")
                conv_sc(W2, A8[:], a4s, a2s, lo, c1, s, "cv2")
                if s == 7:
                    residual_out(3, 0, 37)
                if s in IMG_LAST_SC:
                    i = IMG_LAST_SC[s]
                    if i == 3:
                        residual_out(3, 37, H)
                    else:
                        residual_out(i)

    nc.compile()

    if LDW_REUSE:
        # drop PE weight reloads that repeat the previous load verbatim (the
        # array already holds these weights); only sync-free loads come out
        for blk in nc.main_func.blocks:
            keep, prev = [], None
            for inst in blk.instructions:
                if isinstance(inst, mybir.InstLdweights):
                    w = inst.ins[0]
                    key = (w.memref, w.offset, str(w.ap))
                    si = inst.sync_info
                    clean = si is None or (len(si.on_wait) == 0
                                           and len(si.on_update) == 0)
                    if key == prev and clean:
                        continue
                    prev = key
                elif isinstance(inst, mybir.InstMatmult):
                    wdt = inst.ins[1].dtype
                    if inst.is_transpose or wdt == mybir.dt.float32:
                        prev = None  # self-loading matmul clobbers the array
                keep.append(inst)
            blk.instructions = keep
    return nc


def _consts():
    c = np.zeros((1, 27), np.float32)
    for k in range(9):
        c[0, k] = 2.0 / (NW[k] * NA[k])
        c[0, 9 + k] = 1.0 / NA[k]
        c[0, 18 + k] = float(NW[k])
    return c


def _in_maps(inputs):
    x = np.ascontiguousarray(inputs["x"], dtype=np.float32)
    shared = {
        "conv1_w": np.ascontiguousarray(inputs["conv1_w"], dtype=np.float32),
        "conv2_w": np.ascontiguousarray(inputs["conv2_w"], dtype=np.float32),
        "gamma1": np.ascontiguousarray(inputs["gamma1"], dtype=np.float32),
        "beta1": np.ascontiguousarray(inputs["beta1"], dtype=np.float32),
        "gamma2": np.ascontiguousarray(inputs["gamma2"], dtype=np.float32),
        "beta2": np.ascontiguousarray(inputs["beta2"], dtype=np.float32),
        "p1": np.ascontiguousarray(inputs["p1"], dtype=np.float32),
        "p2": np.ascontiguousarray(inputs["p2"], dtype=np.float32),
        "gn1": np.ascontiguousarray(inputs["gn1"], dtype=np.float32),
        "gn2": np.ascontiguousarray(inputs["gn2"], dtype=np.float32),
        "tau": np.asarray(inputs["tau"], dtype=np.float32).reshape(1),
        "consts": _consts(),
    }
    return [dict(shared, x=x[c * BS:(c + 1) * BS]) for c in range(N_CORES)]


def _get_nc():
    if "nc" not in _CACHE:
        _CACHE["nc"] = _build()
    return _CACHE["nc"]


def _run(in_maps, trace=False):
    nc = _get_nc()
    return bass_utils.run_bass_kernel_spmd(
        nc, in_maps, core_ids=list(range(N_CORES)), trace=trace)


def kernel(**inputs) -> np.ndarray:
    res = _run(_in_maps(inputs))
    return np.concatenate([res.results[c]["out"] for c in range(N_CORES)], axis=0)


# revision 25
# speedup vs baseline: 1.3531x; 1.0013x over previous
"""Trainium2 Bass kernel for DNAS PreBasicBlock (mixed-quantization residual block).

Math:
  out = residual + mixed_qconv2(bn_relu2(mixed_qconv1(bn_relu1(x))))

Key optimizations (see git-less history in comments):
  * relu+clip fold; candidate folding by activation bits (3 convs per layer);
    A_4 = round(A_8/17), A_2 = round(A_4/5) derived on the fly per superchunk.
  * layer-1 weights hi/lo bf16 (near-fp32); layer-2 single fp16 pass (x256).
  * warmup collective at t=0 absorbs the CC engine's one-time algo/library
    setup (~50us) so the BN-stats AllReduce runs at warm latency.
  * x DMA split 16 ways (one queue each); weight DMAs queued right behind
    (2 pieces per candidate), single DMA pass per layer: raws stay resident
    in raw9 and tanh/quant chain runs in place.
  * BN batch stats: DVE computes sums, ACT computes sum-of-squares
    (halves the serial ACT time on the allreduce trigger path).
  * full W1 prep emitted before conv; first superchunk still starts with
    group 2 only so the PE ramps as early as possible.
  * layer-2 stats allreduce split: everything except img3's tail rows goes
    out while superchunk 8 computes; only the tail rides the critical path.
  * first image quantized in two steps (rows 0..29 first) so the first
    superchunk's derive never waits on a full-image pass.
  * conv as 9 shift-matmuls per pass accumulating in PSUM; matmuls reuse
    loaded PE weights across the 3 chunks of a superchunk (ldweights skip).
"""
import sys

sys.path.insert(0, "/opt/trn_rl_repo")

import numpy as np

import concourse.bass as bass
import concourse.tile as tile
from concourse import bacc, bass_utils, mybir

dt = mybir.dt
Alu = mybir.AluOpType
Act = mybir.ActivationFunctionType

N_CORES = 8
B, C, H, W = 32, 128, 56, 56
BS = B // N_CORES          # batch shard per core
HP, WP = H + 2, W + 2      # padded image: 1 row/col of zeros on each side
IMG = HP * WP              # 3364
APIX = BS * IMG            # 13456
BASE = WP + 1              # first valid flat offset within an image: 59
VSTART = BASE
VEND = (BS - 1) * IMG + H * WP + W + 1  # one past last valid: 13397
CHUNK = 512
NCHUNK = -(-(VEND - VSTART) // CHUNK)  # 27
SCCH = 3                   # chunks per superchunk
SLEN = SCCH * CHUNK        # 1536
NSC = -(-NCHUNK // SCCH)   # 9
STG = 1664                 # derive-staging width (halo + alignment slack)
NPIX_IMG = H * W           # 3136
IMG_SPAN = H * WP          # 3248: rows 1..56 as 56 x 58 view
NTOT = float(B * H * W)    # BN divisor 100352
MAGIC = 12582912.0         # 1.5*2^23: fp32 round-to-int via add/sub
MAGICH = 1536.0            # 1.5*2^10: fp16 round-to-int via f32->fp16 convert
EPS = 1e-5
WSCALE = 256.0             # fp16 weight scale (off subnormals)
LDW_REUSE = True           # skip PE weight reload on repeat-lhsT matmuls

BITS = [2, 4, 8]
NW = [2 ** BITS[k // 3] - 1 for k in range(9)]   # weight levels per candidate
NA = [2 ** BITS[k % 3] - 1 for k in range(9)]    # activation levels per candidate
KORDER = [2, 5, 8, 1, 4, 7, 0, 3, 6]             # group-major, ba=8 group first
GROUP_KS = {2: [2, 5, 8], 1: [1, 4, 7], 0: [0, 3, 6]}
TAPS = [(ky - 1) * WP + (kx - 1) for ky in range(3) for kx in range(3)]

# which superchunks become available after which image is quantized:
SC_NEEDS_IMG = []
for _s in range(NSC):
    _hi = min(VSTART + 1536 * (_s + 1) + 59, APIX)
    SC_NEEDS_IMG.append(min((_hi - 1) // IMG, BS - 1))

# image i's last valid pixel lives in superchunk:
IMG_LAST_SC = {}
for _i in range(BS):
    _last = _i * IMG + H * WP + W
    _s = min((_last - VSTART) // SLEN, NSC - 1)
    IMG_LAST_SC[_s] = _i

# rows of img0 needed before superchunk 0 can derive+run: pixels [0, STG)
ROWS_SC0 = -(-(STG - BASE) // WP) + 1  # 29

_CACHE = {}


def _chunks_of_sc(s):
    out = []
    for c in range(SCCH):
        ci = s * SCCH + c
        if ci >= NCHUNK:
            break
        gs = VSTART + ci * CHUNK
        ln = min(CHUNK, VEND - gs)
        out.append((c * CHUNK, gs, ln))
    return out


def _build():
    nc = bacc.Bacc("TRN2", target_bir_lowering=False, debug=False,
                   num_devices=N_CORES)

    x_in = nc.dram_tensor("x", [BS, C, H, W], dt.float32, kind="ExternalInput")
    w1_in = nc.dram_tensor("conv1_w", [9, C, C, 3, 3], dt.float32, kind="ExternalInput")
    w2_in = nc.dram_tensor("conv2_w", [9, C, C, 3, 3], dt.float32, kind="ExternalInput")
    g1_in = nc.dram_tensor("gamma1", [C], dt.float32, kind="ExternalInput")
    b1_in = nc.dram_tensor("beta1", [C], dt.float32, kind="ExternalInput")
    g2_in = nc.dram_tensor("gamma2", [C], dt.float32, kind="ExternalInput")
    b2_in = nc.dram_tensor("beta2", [C], dt.float32, kind="ExternalInput")
    p1_in = nc.dram_tensor("p1", [9], dt.float32, kind="ExternalInput")
    p2_in = nc.dram_tensor("p2", [9], dt.float32, kind="ExternalInput")
    gn1_in = nc.dram_tensor("gn1", [9], dt.float32, kind="ExternalInput")
    gn2_in = nc.dram_tensor("gn2", [9], dt.float32, kind="ExternalInput")
    tau_in = nc.dram_tensor("tau", [1], dt.float32, kind="ExternalInput")
    consts_in = nc.dram_tensor("consts", [1, 27], dt.float32, kind="ExternalInput")
    out_dram = nc.dram_tensor("out", [BS, C, H, W], dt.float32, kind="ExternalOutput")

    from concourse.masks import make_identity

    with tile.TileContext(nc) as tc:
        with tc.tile_pool(name="main", bufs=1) as sb, \
             tc.tile_pool(name="ps", bufs=1, space="PSUM") as ps, \
             tc.tile_pool(name="dram", bufs=1, space="DRAM") as dram:

            # ---------- static tiles / input DMAs ----------
            ident = sb.tile([128, 128], dt.float32)
            make_identity(nc, ident[:])

            # A8 memset is emitted later (after the stats-allreduce trigger) so
            # the gpsimd queue reaches that trigger without an 11us stall
            A8 = sb.tile([C, APIX], dt.float16, tag="A8")

            x_sb = sb.tile([C, BS * NPIX_IMG], dt.float32, tag="big", name="x_sb")
            x_src = x_in.ap().rearrange("b c h w -> c b (h w)")
            QTR = NPIX_IMG // 4
            for i in range(BS):
                for qq in range(4):  # 16 pieces -> all DMA queues
                    lo_ = (i * 4 + qq) * QTR
                    nc.sync.dma_start(
                        x_sb[:, lo_:lo_ + QTR],
                        x_src[:, i, qq * QTR:(qq + 1) * QTR])

            # weight raws: single DMA pass, resident; queued behind x
            raw9 = sb.tile([C, 9 * 1152], dt.float32, tag="raw9", name="raw9")
            wsrc1 = w1_in.ap().rearrange("k o i a b -> k o (i a b)")
            for k in KORDER:
                for hh in range(2):
                    nc.sync.dma_start(
                        raw9[:, k * 1152 + hh * 576:k * 1152 + (hh + 1) * 576],
                        wsrc1[k][:, hh * 576:(hh + 1) * 576])

            def row(name, t, n):
                r = sb.tile([1, n], dt.float32, name=name)
                nc.sync.dma_start(r[:], t.ap()[None, :])
                return r

            p1r = row("p1r", p1_in, 9)
            gn1r = row("gn1r", gn1_in, 9)
            p2r = row("p2r", p2_in, 9)
            gn2r = row("gn2r", gn2_in, 9)
            taur = row("taur", tau_in, 1)
            constsr = sb.tile([1, 27], dt.float32)
            nc.sync.dma_start(constsr[:], consts_in.ap())

            def col128(name, t):
                r = sb.tile([C, 1], dt.float32, name=name)
                nc.sync.dma_start(r[:], t.ap()[:, None])
                return r

            gam1, bet1 = col128("gam1", g1_in), col128("bet1", b1_in)
            gam2, bet2 = col128("gam2", g2_in), col128("bet2", b2_in)

            rtau = sb.tile([1, 1], dt.float32)
            nc.vector.reciprocal(rtau[:], taur[:])

            # ---------- per-layer softmax -> alpha/gamma strip -> broadcast ----------
            def softmax_strip(pr, gnr, tag):
                u = sb.tile([1, 9], dt.float32, name=f"u_{tag}")
                nc.vector.tensor_tensor(u[:], pr[:], gnr[:], Alu.add)
                nc.vector.tensor_scalar(u[:], u[:], rtau[:, 0:1], None, Alu.mult)
                mx = sb.tile([1, 1], dt.float32, name=f"mx_{tag}")
                nc.vector.tensor_reduce(mx[:], u[:], axis=mybir.AxisListType.X,
                                        op=Alu.max)
                nmx = sb.tile([1, 1], dt.float32, name=f"nmx_{tag}")
                nc.vector.tensor_scalar(nmx[:], mx[:], -1.0, None, Alu.mult)
                e = sb.tile([1, 9], dt.float32, name=f"e_{tag}")
                nc.scalar.activation(e[:], u[:], Act.Exp, bias=nmx[:, 0:1], scale=1.0)
                ssum = sb.tile([1, 1], dt.float32, name=f"ss_{tag}")
                nc.vector.tensor_reduce(ssum[:], e[:], axis=mybir.AxisListType.X,
                                        op=Alu.add)
                rsum = sb.tile([1, 1], dt.float32, name=f"rs_{tag}")
                nc.vector.reciprocal(rsum[:], ssum[:])
                wrow = sb.tile([1, 9], dt.float32, name=f"w_{tag}")
                nc.vector.tensor_scalar(wrow[:], e[:], rsum[:, 0:1], None, Alu.mult)
                strip = sb.tile([1, 12], dt.float32, name=f"strip_{tag}")
                nc.vector.tensor_tensor(strip[:, 0:9], wrow[:], constsr[:, 0:9],
                                        Alu.mult)
                pe1 = sb.tile([1, 9], dt.float32, name=f"pe1_{tag}")
                nc.vector.tensor_tensor(pe1[:], wrow[:], constsr[:, 9:18], Alu.mult)
                pe13 = pe1[:].rearrange("p (i g) -> p i g", g=3)
                for g in range(3):
                    nc.vector.tensor_reduce(strip[:, 9 + g:10 + g], pe13[:, :, g],
                                            axis=mybir.AxisListType.X, op=Alu.add,
                                            negate=True)
                bcast = sb.tile([C, 12], dt.float32, name=f"bcast_{tag}")
                nc.gpsimd.partition_broadcast(bcast[:], strip[:])
                return bcast

            # ---------- BN batch stats: DVE sums, ACT sum-of-squares ----------
            def img_stats(src3d, stats_cols, col, tag, src2d=None):
                a = src3d.shape[1]
                if src2d is not None:  # contiguous image: one-shot row reduce
                    nc.vector.tensor_reduce(stats_cols[:, col:col + 1], src2d,
                                            axis=mybir.AxisListType.X, op=Alu.add)
                else:  # strided view: reduce innermost, then the row of rows
                    rs = sb.tile([C, H], dt.float32, tag="rsum", bufs=2,
                                 name=f"rs_{tag}_{col}")
                    nc.vector.tensor_reduce(rs[:, 0:a], src3d,
                                            axis=mybir.AxisListType.X, op=Alu.add)
                    nc.vector.tensor_reduce(stats_cols[:, col:col + 1], rs[:, 0:a],
                                            axis=mybir.AxisListType.X, op=Alu.add)
                n = src3d.shape[1] * src3d.shape[2]
                scr2 = sb.tile([C, NPIX_IMG], dt.float32, tag="scr", bufs=2,
                               name=f"scq_{tag}_{col}")
                scr23 = scr2[:, 0:n].rearrange("p (a b) -> p a b",
                                               b=src3d.shape[2])
                nc.scalar.activation(scr23, src3d, Act.Square, bias=0.0, scale=1.0,
                                     accum_out=stats_cols[:, 5 + col:6 + col])

            def allreduce_cols(loc, tag):
                cin = dram.tile([C, 2], dt.float32, name=f"ccin_{tag}")
                cout = dram.tile([C, 2], dt.float32, addr_space="Shared",
                                 name=f"ccout_{tag}")
                nc.sync.dma_start(cin[:], loc[:])
                nc.gpsimd.collective_compute(
                    "AllReduce", Alu.add,
                    replica_groups=[list(range(N_CORES))],
                    ins=[cin.opt()], outs=[cout.opt()])
                glob = sb.tile([C, 2], dt.float32, name=f"glob_{tag}")
                nc.sync.dma_start(glob[:], cout[:])
                return glob

            def reduce_stats(stats_cols, tag, lo=0, hi=5):
                loc = sb.tile([C, 2], dt.float32, name=f"loc_{tag}")
                sc3 = stats_cols[:].rearrange("p (s i) -> p s i", s=2)
                nc.vector.tensor_reduce(loc[:], sc3[:, :, lo:hi],
                                        axis=mybir.AxisListType.X, op=Alu.add)
                return loc

            def bn_scalars(glob, gam, bet, tag):
                def t1(name):
                    return sb.tile([C, 1], dt.float32, name=f"{name}_{tag}")
                mean, e2, msq, var, ve = (t1("mean"), t1("e2"), t1("msq"),
                                          t1("var"), t1("ve"))
                nc.vector.tensor_scalar(mean[:], glob[:, 0:1], 1.0 / NTOT, None,
                                        Alu.mult)
                nc.vector.tensor_scalar(e2[:], glob[:, 1:2], 1.0 / NTOT, None,
                                        Alu.mult)
                nc.vector.tensor_tensor(msq[:], mean[:], mean[:], Alu.mult)
                nc.vector.tensor_tensor(var[:], e2[:], msq[:], Alu.subtract)
                nc.vector.tensor_scalar(ve[:], var[:], EPS, None, Alu.add)
                sq, y = t1("sq"), t1("y0")
                nc.scalar.activation(sq[:], ve[:], Act.Sqrt, bias=0.0, scale=1.0)
                nc.vector.reciprocal(y[:], sq[:])
                for it in range(2):  # Newton: y <- y*(1.5 - 0.5*ve*y^2)
                    tt1, tt2, tt3, yn = (t1(f"n{it}a"), t1(f"n{it}b"),
                                         t1(f"n{it}c"), t1(f"y{it + 1}"))
                    nc.vector.tensor_tensor(tt1[:], y[:], y[:], Alu.mult)
                    nc.vector.tensor_tensor(tt2[:], tt1[:], ve[:], Alu.mult)
                    nc.vector.tensor_scalar(tt3[:], tt2[:], -0.5, 1.5, Alu.mult,
                                            Alu.add)
                    nc.vector.tensor_tensor(yn[:], y[:], tt3[:], Alu.mult)
                    y = yn
                sbn, bt, sq_, bq_ = (t1("sbn"), t1("bt"), t1("sclq"), t1("biasq"))
                nc.vector.tensor_tensor(sbn[:], gam[:], y[:], Alu.mult)
                nc.vector.tensor_tensor(bt[:], mean[:], sbn[:], Alu.mult)
                # u = 255*(s*x + b): sq = 255*s ; bq = 255*(beta - mean*s)
                nc.vector.tensor_scalar(bq_[:], bt[:], -255.0, bet255(bet, tag),
                                        Alu.mult, Alu.add)
                nc.vector.tensor_scalar(sq_[:], sbn[:], 255.0, None, Alu.mult)
                return sq_, bq_

            _bet255 = {}

            def bet255(bet, tag):
                if tag not in _bet255:
                    b = sb.tile([C, 1], dt.float32, name=f"bet255_{tag}")
                    nc.vector.tensor_scalar(b[:], bet[:], 255.0, None, Alu.mult)
                    _bet255[tag] = b
                return _bet255[tag][:, 0:1]

            # ---------- quantize rows [r0, r1) of one image into the 8-bit grid ----------
            def quantize_img(src3d_full, A8t, sq_, bq_, i, tag, r0=0, r1=H):
                # u = relu(255*(s*x+b)); round+clamp: (min(u,255)+M)-M
                src3d = src3d_full[:, r0:r1]
                n = (r1 - r0) * W
                u = sb.tile([C, NPIX_IMG], dt.float32, tag="scr", bufs=2,
                            name=f"qu_{tag}_{i}_{r0}")
                u3 = u[:, 0:n].rearrange("p (a b) -> p a b", a=r1 - r0)
                nc.scalar.activation(u3, src3d, Act.Relu, bias=bq_[:, 0:1],
                                     scale=sq_[:, 0:1])
                nc.vector.tensor_scalar(u[:, 0:n], u[:, 0:n], 255.0, MAGIC,
                                        Alu.min, Alu.add)
                dst = A8t[:, i * IMG + BASE + r0 * WP:
                          i * IMG + BASE + r1 * WP]
                dst3 = dst.rearrange("p (a b) -> p a b", b=WP)[:, :, 0:W]
                nc.vector.tensor_scalar(dst3, u3, MAGIC, None, Alu.subtract)

            # ---------- weight preparation ----------
            def prep_amax(tag):
                # partition-axis max via PE transpose + DVE reduce + tiny DMA
                amax = sb.tile([C, 9], dt.float32, name=f"amax_{tag}")
                for k in KORDER:
                    nc.vector.tensor_reduce(amax[:, k:k + 1],
                                            raw9[:, k * 1152:(k + 1) * 1152],
                                            axis=mybir.AxisListType.X, op=Alu.max,
                                            apply_absolute_value=True)
                tp9 = ps.tile([9, 128], dt.float32, tag="tps", bufs=2,
                              name=f"tp9_{tag}")
                nc.tensor.transpose(tp9[:], amax[:], ident[:])
                mx9 = sb.tile([9, 1], dt.float32, name=f"mx9_{tag}")
                nc.vector.tensor_reduce(mx9[:], tp9[:], axis=mybir.AxisListType.X,
                                        op=Alu.max)
                mrow = sb.tile([1, 9], dt.float32, name=f"mrow_{tag}")
                for k in range(9):
                    nc.sync.dma_start(mrow[0:1, k:k + 1], mx9[k:k + 1, 0:1])
                tam = sb.tile([1, 9], dt.float32, name=f"tam_{tag}")
                nc.scalar.activation(tam[:], mrow[:], Act.Tanh, bias=0.0, scale=1.0)
                a2 = sb.tile([1, 9], dt.float32, name=f"a2_{tag}")
                nc.vector.tensor_scalar(a2[:], tam[:], 2.0, None, Alu.mult)
                r2r = sb.tile([1, 9], dt.float32, name=f"r2r_{tag}")
                nc.vector.reciprocal(r2r[:], a2[:])
                r2 = sb.tile([C, 9], dt.float32, name=f"r2_{tag}")
                nc.gpsimd.partition_broadcast(r2[:], r2r[:])
                return r2

            def prep_tanh(tag):
                # tanh in place over the resident raws (after amax extraction)
                for k in KORDER:
                    v = raw9[:, k * 1152:(k + 1) * 1152]
                    nc.scalar.activation(v, v, Act.Tanh, bias=0.0, scale=1.0)

            def prep_chain(r2, bcast, tag, g):
                """accumulate one ba-group's quantized candidates -> wacc."""
                wacc = None
                for pos, k in enumerate(GROUP_KS[g]):
                    th = raw9[:, k * 1152:(k + 1) * 1152]
                    # wn = th/(2amax)+0.5 ; u2 = wn*nw + M (rounds) ; m = u2-M
                    # (+0.5*nw must NOT fold into M: M+0.5nw isn't fp32-exact)
                    nc.vector.tensor_scalar(th, th, r2[:, k:k + 1], 0.5,
                                            Alu.mult, Alu.add)
                    nc.vector.tensor_scalar(th, th, float(NW[k]), MAGIC,
                                            Alu.mult, Alu.add)
                    nc.vector.tensor_scalar(th, th, MAGIC, None,
                                            Alu.subtract)
                    if pos == 0:
                        wacc = sb.tile([C, 1152], dt.float32, tag="wacc",
                                       bufs=2, name=f"wacc_{tag}_{g}_{pos}")
                        nc.vector.tensor_scalar(wacc[:], th, bcast[:, k:k + 1],
                                                bcast[:, 9 + g:10 + g],
                                                Alu.mult, Alu.add)
                    else:
                        nxt = sb.tile([C, 1152], dt.float32, tag="wacc",
                                      bufs=2, name=f"wacc_{tag}_{g}_{pos}")
                        nc.vector.scalar_tensor_tensor(nxt[:], th,
                                                       bcast[:, k:k + 1], wacc[:],
                                                       Alu.mult, Alu.add)
                        wacc = nxt
                return wacc

            def prep_transpose(wacc, tag, g, Wt):
                w3 = wacc[:].rearrange("p (i t) -> p i t", t=9)
                for t in range(9):
                    tp = ps.tile([128, 128], dt.float32, tag="tps", bufs=2,
                                 name=f"tp_{tag}_{g}_{t}")
                    nc.tensor.transpose(tp[:], w3[:, :, t], ident[:])
                    nc.scalar.activation(Wt[g][:, t, :], tp[:], Act.Copy,
                                         bias=0.0, scale=WSCALE)

            def alloc_W(tag):
                # fp16 x256: single-pass weights at 16-bit PE streaming rate
                return [sb.tile([C, 9, C], dt.float16, name=f"W_{tag}_{g}")
                        for g in range(3)]

            # ---------- conv pieces ----------
            def derive_sc(src8, s, tag):
                start = VSTART + s * SLEN
                lo = min(max((start - 64) & ~1, 0), APIX - STG)
                a4s = sb.tile([C, STG], dt.float16, tag="a4s", bufs=3,
                              name=f"a4_{tag}_{s}")
                nc.vector.tensor_scalar(a4s[:], src8[:, lo:lo + STG], 1.0 / 17.0,
                                        MAGICH, Alu.mult, Alu.add)
                nc.vector.tensor_scalar(a4s[:], a4s[:], MAGICH, None, Alu.subtract)
                a2s = sb.tile([C, STG], dt.float16, tag="a2s", bufs=3,
                              name=f"a2_{tag}_{s}")
                nc.vector.tensor_scalar(a2s[:], a4s[:], 1.0 / 5.0, MAGICH,
                                        Alu.mult, Alu.add)
                nc.vector.tensor_scalar(a2s[:], a2s[:], MAGICH, None, Alu.subtract)
                return a4s, a2s, lo

            def conv_sc(Wt, src8, a4s, a2s, lo, cdst, s, tag, groups=None):
                """emit conv passes for superchunk s; groups=None -> all."""
                chunks = _chunks_of_sc(s)
                start = VSTART + s * SLEN
                all_passes = [2, 1, 0]
                passes = [(pi, g) for pi, g in enumerate(all_passes)
                          if groups is None or g in groups]
                pt = _sc_psum(tag, s)
                for pi, g in passes:
                    for t in range(9):
                        off = TAPS[t]
                        for ci, (pcol, gs, ln) in enumerate(chunks):
                            if g == 2:
                                rhs = src8[:, gs + off:gs + off + ln]
                            elif g == 1:
                                rhs = a4s[:, gs + off - lo:gs + off - lo + ln]
                            else:
                                rhs = a2s[:, gs + off - lo:gs + off - lo + ln]
                            nc.tensor.matmul(
                                pt[:, pcol:pcol + ln], Wt[g][:, t, :], rhs,
                                start=(pi == 0 and t == 0),
                                stop=(pi == len(all_passes) - 1 and t == 8))
                if groups is None or all_passes[-1] in groups:
                    sc_end = min(start + SLEN, VEND)
                    nc.scalar.activation(cdst[:, start:sc_end],
                                         pt[:, 0:sc_end - start], Act.Copy,
                                         bias=0.0, scale=1.0 / WSCALE)

            _psums = {}

            def _sc_psum(tag, s):
                key = (tag, s)
                if key not in _psums:
                    _psums[key] = ps.tile([128, SLEN], dt.float32, tag="cps",
                                          bufs=2, name=f"ps_{tag}_{s}")
                return _psums[key]

            # ================= LAYER 1 =================
            stats1 = sb.tile([C, 10], dt.float32)
            nc.vector.memset(stats1[:], 0.0)
            x3 = x_sb[:].rearrange("p (b a w) -> p b a w", b=BS, a=H)
            for i in range(BS):
                img_stats(x3[:, i], stats1, i, "s1",
                          src2d=x_sb[:, i * NPIX_IMG:(i + 1) * NPIX_IMG])

            loc1 = reduce_stats(stats1, "c1")
            glob1 = allreduce_cols(loc1, "c1")

            # DVE memset: keeps the gpsimd queue clear so the preamble
            # barrier collective triggers immediately
            nc.vector.memset(A8[:], 0.0)

            bc1 = softmax_strip(p1r, gn1r, "l1")
            bc2 = softmax_strip(p2r, gn2r, "l2")

            r2_1 = prep_amax("w1")
            prep_tanh("w1")

            W1 = alloc_W("w1")
            c1 = sb.tile([C, APIX], dt.float32, tag="big", name="c1buf")
            stats2 = sb.tile([C, 10], dt.float32)
            nc.vector.memset(stats2[:], 0.0)

            # all weight prep happens before the BN-gated quantize so the
            # PE transposes' ACT copies never queue behind the glob1 wait
            for g in (2, 1, 0):
                wacc = prep_chain(r2_1, bc1, "w1", g)
                prep_transpose(wacc, "w1", g, W1)
            nsq1, nbq1 = bn_scalars(glob1, gam1, bet1, "bn1")
            quantize_img(x3[:, 0], A8, nsq1, nbq1, 0, "q1", 0, ROWS_SC0)
            a4s0, a2s0, lo0 = derive_sc(A8[:], 0, "cv1")
            quantize_img(x3[:, 0], A8, nsq1, nbq1, 0, "q1", ROWS_SC0, H)
            quantize_img(x3[:, 1], A8, nsq1, nbq1, 1, "q1")
            quantize_img(x3[:, 2], A8, nsq1, nbq1, 2, "q1")
            quantize_img(x3[:, 3], A8, nsq1, nbq1, 3, "q1")
            # NOTE: all of x must be consumed (quantized) before conv1's first
            # PSUM copy writes c1 -- they share one SBUF slot and the slot
            # handover is tile-granular.

            def c1_img3d(i, r0=0, r1=H):
                off = i * IMG + BASE + r0 * WP
                v = c1[:, off:off + (r1 - r0) * WP]
                return v.rearrange("p (a b) -> p a b", b=WP)[:, :, 0:W]

            conv_sc(W1, A8[:], a4s0, a2s0, lo0, c1, 0, "cv1")
            a4s1, a2s1, lo1 = derive_sc(A8[:], 1, "cv1")
            conv_sc(W1, A8[:], a4s1, a2s1, lo1, c1, 1, "cv1")

            def cv1_after_sc(s):
                if s == 7:  # partial img3 stats (rows 0..37 available)
                    img_stats(c1_img3d(3, 0, 37), stats2, 3, "s2")
                    loc2a = reduce_stats(stats2, "c2a", 0, 4)
                    return allreduce_cols(loc2a, "c2a")
                if s in IMG_LAST_SC:
                    i = IMG_LAST_SC[s]
                    if i != 3:
                        img_stats(c1_img3d(i), stats2, i, "s2")
                return None

            cv1_after_sc(0), cv1_after_sc(1)
            glob2a = None
            w2src = w2_in.ap().rearrange("k o i a b -> k o (i a b)")
            for s in range(2, NSC):
                a4s, a2s, lo = derive_sc(A8[:], s, "cv1")
                conv_sc(W1, A8[:], a4s, a2s, lo, c1, s, "cv1")
                ret = cv1_after_sc(s)
                if ret is not None:
                    glob2a = ret
                if s == 2:
                    # layer-2 raws reuse the raw9 slot once layer-1 chains
                    # are consumed; spread across conv1 superchunks
                    for k in KORDER:
                        for hh in range(2):
                            nc.sync.dma_start(
                                raw9[:, k * 1152 + hh * 576:
                                     k * 1152 + (hh + 1) * 576],
                                w2src[k][:, hh * 576:(hh + 1) * 576])
                elif s == 3:
                    r2_2 = prep_amax("w2")
                    prep_tanh("w2")
                    W2 = alloc_W("w2")
                    wacc_g = prep_chain(r2_2, bc2, "w2", 2)
                    prep_transpose(wacc_g, "w2", 2, W2)
                elif s == 4:
                    wacc_g = prep_chain(r2_2, bc2, "w2", 1)
                    prep_transpose(wacc_g, "w2", 1, W2)
                elif s == 5:
                    wacc_g = prep_chain(r2_2, bc2, "w2", 0)
                    prep_transpose(wacc_g, "w2", 0, W2)

            # ================= LAYER 2 =================
            # tail stats (img3 rows 37..56) -> tiny allreduce; everything else
            # went out during superchunk 8
            img_stats(c1_img3d(3, 37, H), stats2, 4, "s2t")
            loc2b = reduce_stats(stats2, "c2b", 4, 5)
            glob2b = allreduce_cols(loc2b, "c2b")
            glob2 = sb.tile([C, 2], dt.float32, name="glob2")
            nc.vector.tensor_tensor(glob2[:], glob2a[:], glob2b[:], Alu.add)
            nsq2, nbq2 = bn_scalars(glob2, gam2, bet2, "bn2")

            # layer 2 reuses the A8 grid tile (conv1 is done with it)
            out_v = out_dram.ap().rearrange("b c h w -> c b (h w)")

            def residual_out(i, r0=0, r1=H):
                n = (r1 - r0) * W
                xr = sb.tile([C, NPIX_IMG], dt.float32, tag="scr", bufs=2,
                             name=f"xr_{i}_{r0}")
                nc.sync.dma_start(xr[:, 0:n], x_src[:, i, r0 * W:r1 * W])
                xr3 = xr[:, 0:n].rearrange("p (a b) -> p a b", a=r1 - r0)
                nc.vector.tensor_tensor(xr3, xr3, c1_img3d(i, r0, r1), Alu.add)
                nc.sync.dma_start(out_v[:, i, r0 * W:r1 * W], xr[:, 0:n])

            quantize_img(c1_img3d(0), A8, nsq2, nbq2, 0, "q2", 0, ROWS_SC0)
            _q2_done = {}
            for s in range(0, NSC):
                need = SC_NEEDS_IMG[s]
                for i in range(BS):
                    if need >= i and not _q2_done.get(i):
                        if i == 0:
                            quantize_img(c1_img3d(0), A8, nsq2, nbq2, 0, "q2",
                                         ROWS_SC0, H)
                        else:
                            quantize_img(c1_img3d(i), A8, nsq2, nbq2, i, "q2")
                        _q2_done[i] = True
                a4s, a2s, lo = derive_sc(A8[:], s, "cv2")
                conv_sc(W2, A8[:], a4s, a2s, lo, c1, s, "cv2")
                if s == 7:
                    residual_out(3, 0, 37)
                if s in IMG_LAST_SC:
                    i = IMG_LAST_SC[s]
                    if i == 3:
                        residual_out(3, 37, H)
                    else:
                        residual_out(i)

    nc.compile()

    if LDW_REUSE:
        # drop PE weight reloads that repeat the previous load verbatim (the
        # array already holds these weights); only sync-free loads come out
        for blk in nc.main_func.blocks:
            keep, prev = [], None
            for inst in blk.instructions:
                if isinstance(inst, mybir.InstLdweights):
                    w = inst.ins[0]
                    key = (w.memref, w.offset, str(w.ap))
                    si = inst.sync_info
                    clean = si is None or (len(si.on_wait) == 0
                                           and len(si.on_update) == 0)
                    if key == prev and clean:
                        continue
                    prev = key
                elif isinstance(inst, mybir.InstMatmult):
                    wdt = inst.ins[1].dtype
                    if inst.is_transpose or wdt == mybir.dt.float32:
                        prev = None  # self-loading matmul clobbers the array
                keep.append(inst)
            blk.instructions = keep
    return nc


def _consts():
    c = np.zeros((1, 27), np.float32)
    for k in range(9):
        c[0, k] = 2.0 / (NW[k] * NA[k])
        c[0, 9 + k] = 1.0 / NA[k]
        c[0, 18 + k] = float(NW[k])
    return c


def _in_maps(inputs):
    x = np.ascontiguousarray(inputs["x"], dtype=np.float32)
    shared = {
        "conv1_w": np.ascontiguousarray(inputs["conv1_w"], dtype=np.float32),
        "conv2_w": np.ascontiguousarray(inputs["conv2_w"], dtype=np.float32),
        "gamma1": np.ascontiguousarray(inputs["gamma1"], dtype=np.float32),
        "beta1": np.ascontiguousarray(inputs["beta1"], dtype=np.float32),
        "gamma2": np.ascontiguousarray(inputs["gamma2"], dtype=np.float32),
        "beta2": np.ascontiguousarray(inputs["beta2"], dtype=np.float32),
        "p1": np.ascontiguousarray(inputs["p1"], dtype=np.float32),
        "p2": np.ascontiguousarray(inputs["p2"], dtype=np.float32),
        "gn1": np.ascontiguousarray(inputs["gn1"], dtype=np.float32),
        "gn2": np.ascontiguousarray(inputs["gn2"], dtype=np.float32),
        "tau": np.asarray(inputs["tau"], dtype=np.float32).reshape(1),
        "consts": _consts(),
    }
    return [dict(shared, x=x[c * BS:(c + 1) * BS]) for c in range(N_CORES)]


def _get_nc():
    if "nc" not in _CACHE:
        _CACHE["nc"] = _build()
    return _CACHE["nc"]


def _run(in_maps, trace=False):
    nc = _get_nc()
    return bass_utils.run_bass_kernel_spmd(
        nc, in_maps, core_ids=list(range(N_CORES)), trace=trace)


def kernel(**inputs) -> np.ndarray:
    res = _run(_in_maps(inputs))
    return np.concatenate([res.results[c]["out"] for c in range(N_CORES)], axis=0)


# revision 35
# speedup vs baseline: 1.3537x; 1.0005x over previous
"""Trainium2 Bass kernel for DNAS PreBasicBlock (mixed-quantization residual block).

Math:
  out = residual + mixed_qconv2(bn_relu2(mixed_qconv1(bn_relu1(x))))

Key optimizations (see git-less history in comments):
  * relu+clip fold; candidate folding by activation bits (3 convs per layer);
    A_4 = round(A_8/17), A_2 = round(A_4/5) derived on the fly per superchunk.
  * layer-1 weights hi/lo bf16 (near-fp32); layer-2 single fp16 pass (x256).
  * warmup collective at t=0 absorbs the CC engine's one-time algo/library
    setup (~50us) so the BN-stats AllReduce runs at warm latency.
  * x DMA split 16 ways (one queue each); weight DMAs queued right behind
    (2 pieces per candidate), single DMA pass per layer: raws stay resident
    in raw9 and tanh/quant chain runs in place.
  * BN batch stats: DVE computes sums, ACT computes sum-of-squares
    (halves the serial ACT time on the allreduce trigger path).
  * full W1 prep emitted before conv; first superchunk still starts with
    group 2 only so the PE ramps as early as possible.
  * layer-2 stats allreduce split: everything except img3's tail rows goes
    out while superchunk 8 computes; only the tail rides the critical path.
  * first image quantized in two steps (rows 0..29 first) so the first
    superchunk's derive never waits on a full-image pass.
  * conv as 9 shift-matmuls per pass accumulating in PSUM; matmuls reuse
    loaded PE weights across the 3 chunks of a superchunk (ldweights skip).
"""
import sys

sys.path.insert(0, "/opt/trn_rl_repo")

import numpy as np

import concourse.bass as bass
import concourse.tile as tile
from concourse import bacc, bass_utils, mybir

dt = mybir.dt
Alu = mybir.AluOpType
Act = mybir.ActivationFunctionType

N_CORES = 8
B, C, H, W = 32, 128, 56, 56
BS = B // N_CORES          # batch shard per core
HP, WP = H + 2, W + 2      # padded image: 1 row/col of zeros on each side
IMG = HP * WP              # 3364
APIX = BS * IMG            # 13456
BASE = WP + 1              # first valid flat offset within an image: 59
VSTART = BASE
VEND = (BS - 1) * IMG + H * WP + W + 1  # one past last valid: 13397
CHUNK = 512
NCHUNK = -(-(VEND - VSTART) // CHUNK)  # 27
SCCH = 3                   # chunks per superchunk
SLEN = SCCH * CHUNK        # 1536
NSC = -(-NCHUNK // SCCH)   # 9
STG = 1664                 # derive-staging width (halo + alignment slack)
NPIX_IMG = H * W           # 3136
IMG_SPAN = H * WP          # 3248: rows 1..56 as 56 x 58 view
NTOT = float(B * H * W)    # BN divisor 100352
MAGIC = 12582912.0         # 1.5*2^23: fp32 round-to-int via add/sub
MAGICH = 1536.0            # 1.5*2^10: fp16 round-to-int via f32->fp16 convert
EPS = 1e-5
WSCALE = 256.0             # fp16 weight scale (off subnormals)
LDW_REUSE = True           # skip PE weight reload on repeat-lhsT matmuls

BITS = [2, 4, 8]
NW = [2 ** BITS[k // 3] - 1 for k in range(9)]   # weight levels per candidate
NA = [2 ** BITS[k % 3] - 1 for k in range(9)]    # activation levels per candidate
KORDER = [2, 5, 8, 1, 4, 7, 0, 3, 6]             # group-major, ba=8 group first
GROUP_KS = {2: [2, 5, 8], 1: [1, 4, 7], 0: [0, 3, 6]}
TAPS = [(ky - 1) * WP + (kx - 1) for ky in range(3) for kx in range(3)]

# which superchunks become available after which image is quantized:
SC_NEEDS_IMG = []
for _s in range(NSC):
    _hi = min(VSTART + 1536 * (_s + 1) + 59, APIX)
    SC_NEEDS_IMG.append(min((_hi - 1) // IMG, BS - 1))

# image i's last valid pixel lives in superchunk:
IMG_LAST_SC = {}
for _i in range(BS):
    _last = _i * IMG + H * WP + W
    _s = min((_last - VSTART) // SLEN, NSC - 1)
    IMG_LAST_SC[_s] = _i

# img3 row ranges completed by each chunk of the final superchunk
_IMG3_BASE = (BS - 1) * IMG + BASE
TAIL_PIECES = []
_r0 = 37
for _c in range(SCCH):
    _ci = (NSC - 1) * SCCH + _c
    if _ci >= NCHUNK:
        break
    _gs = VSTART + _ci * CHUNK
    _ln = min(CHUNK, VEND - _gs)
    _r1 = min((_gs + _ln - _IMG3_BASE - W) // WP + 1, H)
    TAIL_PIECES.append((_r0, _r1))
    _r0 = _r1

# rows of img0 needed before superchunk 0 can derive+run: pixels [0, STG)
ROWS_SC0 = -(-(STG - BASE) // WP) + 1  # 29

_CACHE = {}


def _chunks_of_sc(s):
    out = []
    for c in range(SCCH):
        ci = s * SCCH + c
        if ci >= NCHUNK:
            break
        gs = VSTART + ci * CHUNK
        ln = min(CHUNK, VEND - gs)
        out.append((c * CHUNK, gs, ln))
    return out


def _build():
    nc = bacc.Bacc("TRN2", target_bir_lowering=False, debug=False,
                   num_devices=N_CORES)

    x_in = nc.dram_tensor("x", [BS, C, H, W], dt.float32, kind="ExternalInput")
    w1_in = nc.dram_tensor("conv1_w", [9, C, C, 3, 3], dt.float32, kind="ExternalInput")
    w2_in = nc.dram_tensor("conv2_w", [9, C, C, 3, 3], dt.float32, kind="ExternalInput")
    g1_in = nc.dram_tensor("gamma1", [C], dt.float32, kind="ExternalInput")
    b1_in = nc.dram_tensor("beta1", [C], dt.float32, kind="ExternalInput")
    g2_in = nc.dram_tensor("gamma2", [C], dt.float32, kind="ExternalInput")
    b2_in = nc.dram_tensor("beta2", [C], dt.float32, kind="ExternalInput")
    p1_in = nc.dram_tensor("p1", [9], dt.float32, kind="ExternalInput")
    p2_in = nc.dram_tensor("p2", [9], dt.float32, kind="ExternalInput")
    gn1_in = nc.dram_tensor("gn1", [9], dt.float32, kind="ExternalInput")
    gn2_in = nc.dram_tensor("gn2", [9], dt.float32, kind="ExternalInput")
    tau_in = nc.dram_tensor("tau", [1], dt.float32, kind="ExternalInput")
    consts_in = nc.dram_tensor("consts", [1, 27], dt.float32, kind="ExternalInput")
    out_dram = nc.dram_tensor("out", [BS, C, H, W], dt.float32, kind="ExternalOutput")

    from concourse.masks import make_identity

    with tile.TileContext(nc) as tc:
        with tc.tile_pool(name="main", bufs=1) as sb, \
             tc.tile_pool(name="ps", bufs=1, space="PSUM") as ps, \
             tc.tile_pool(name="dram", bufs=1, space="DRAM") as dram:

            # ---------- static tiles / input DMAs ----------
            ident = sb.tile([128, 128], dt.float32)
            make_identity(nc, ident[:])

            # A8 memset is emitted later (after the stats-allreduce trigger) so
            # the gpsimd queue reaches that trigger without an 11us stall
            A8 = sb.tile([C, APIX], dt.float16, tag="A8")

            x_sb = sb.tile([C, BS * NPIX_IMG], dt.float32, tag="big", name="x_sb")
            x_src = x_in.ap().rearrange("b c h w -> c b (h w)")
            QTR = NPIX_IMG // 4
            for i in range(BS):
                for qq in range(4):  # 16 pieces -> all DMA queues
                    lo_ = (i * 4 + qq) * QTR
                    nc.sync.dma_start(
                        x_sb[:, lo_:lo_ + QTR],
                        x_src[:, i, qq * QTR:(qq + 1) * QTR])

            # weight raws: single DMA pass, resident; queued behind x
            raw9 = sb.tile([C, 9 * 1152], dt.float32, tag="raw9", name="raw9")
            wsrc1 = w1_in.ap().rearrange("k o i a b -> k o (i a b)")
            for k in KORDER:
                for hh in range(2):
                    nc.sync.dma_start(
                        raw9[:, k * 1152 + hh * 576:k * 1152 + (hh + 1) * 576],
                        wsrc1[k][:, hh * 576:(hh + 1) * 576])

            def row(name, t, n):
                r = sb.tile([1, n], dt.float32, name=name)
                nc.sync.dma_start(r[:], t.ap()[None, :])
                return r

            p1r = row("p1r", p1_in, 9)
            gn1r = row("gn1r", gn1_in, 9)
            p2r = row("p2r", p2_in, 9)
            gn2r = row("gn2r", gn2_in, 9)
            taur = row("taur", tau_in, 1)
            constsr = sb.tile([1, 27], dt.float32)
            nc.sync.dma_start(constsr[:], consts_in.ap())

            def col128(name, t):
                r = sb.tile([C, 1], dt.float32, name=name)
                nc.sync.dma_start(r[:], t.ap()[:, None])
                return r

            gam1, bet1 = col128("gam1", g1_in), col128("bet1", b1_in)
            gam2, bet2 = col128("gam2", g2_in), col128("bet2", b2_in)

            rtau = sb.tile([1, 1], dt.float32)
            nc.vector.reciprocal(rtau[:], taur[:])

            # ---------- per-layer softmax -> alpha/gamma strip -> broadcast ----------
            def softmax_strip(pr, gnr, tag):
                u = sb.tile([1, 9], dt.float32, name=f"u_{tag}")
                nc.vector.tensor_tensor(u[:], pr[:], gnr[:], Alu.add)
                nc.vector.tensor_scalar(u[:], u[:], rtau[:, 0:1], None, Alu.mult)
                mx = sb.tile([1, 1], dt.float32, name=f"mx_{tag}")
                nc.vector.tensor_reduce(mx[:], u[:], axis=mybir.AxisListType.X,
                                        op=Alu.max)
                nmx = sb.tile([1, 1], dt.float32, name=f"nmx_{tag}")
                nc.vector.tensor_scalar(nmx[:], mx[:], -1.0, None, Alu.mult)
                e = sb.tile([1, 9], dt.float32, name=f"e_{tag}")
                nc.scalar.activation(e[:], u[:], Act.Exp, bias=nmx[:, 0:1], scale=1.0)
                ssum = sb.tile([1, 1], dt.float32, name=f"ss_{tag}")
                nc.vector.tensor_reduce(ssum[:], e[:], axis=mybir.AxisListType.X,
                                        op=Alu.add)
                rsum = sb.tile([1, 1], dt.float32, name=f"rs_{tag}")
                nc.vector.reciprocal(rsum[:], ssum[:])
                wrow = sb.tile([1, 9], dt.float32, name=f"w_{tag}")
                nc.vector.tensor_scalar(wrow[:], e[:], rsum[:, 0:1], None, Alu.mult)
                strip = sb.tile([1, 12], dt.float32, name=f"strip_{tag}")
                nc.vector.tensor_tensor(strip[:, 0:9], wrow[:], constsr[:, 0:9],
                                        Alu.mult)
                pe1 = sb.tile([1, 9], dt.float32, name=f"pe1_{tag}")
                nc.vector.tensor_tensor(pe1[:], wrow[:], constsr[:, 9:18], Alu.mult)
                pe13 = pe1[:].rearrange("p (i g) -> p i g", g=3)
                for g in range(3):
                    nc.vector.tensor_reduce(strip[:, 9 + g:10 + g], pe13[:, :, g],
                                            axis=mybir.AxisListType.X, op=Alu.add,
                                            negate=True)
                bcast = sb.tile([C, 12], dt.float32, name=f"bcast_{tag}")
                nc.gpsimd.partition_broadcast(bcast[:], strip[:])
                return bcast

            # ---------- BN batch stats: DVE sums, ACT sum-of-squares ----------
            STATW = 7  # sums in cols 0..6, sum-of-squares in 7..13

            def img_stats(src3d, stats_cols, col, tag, src2d=None):
                a = src3d.shape[1]
                if src2d is not None:  # contiguous image: one-shot row reduce
                    nc.vector.tensor_reduce(stats_cols[:, col:col + 1], src2d,
                                            axis=mybir.AxisListType.X, op=Alu.add)
                else:  # strided view: reduce innermost, then the row of rows
                    rs = sb.tile([C, H], dt.float32, tag="rsum", bufs=2,
                                 name=f"rs_{tag}_{col}")
                    nc.vector.tensor_reduce(rs[:, 0:a], src3d,
                                            axis=mybir.AxisListType.X, op=Alu.add)
                    nc.vector.tensor_reduce(stats_cols[:, col:col + 1], rs[:, 0:a],
                                            axis=mybir.AxisListType.X, op=Alu.add)
                n = src3d.shape[1] * src3d.shape[2]
                scr2 = sb.tile([C, NPIX_IMG], dt.float32, tag="scr", bufs=2,
                               name=f"scq_{tag}_{col}")
                scr23 = scr2[:, 0:n].rearrange("p (a b) -> p a b",
                                               b=src3d.shape[2])
                nc.scalar.activation(scr23, src3d, Act.Square, bias=0.0, scale=1.0,
                                     accum_out=stats_cols[:, STATW + col:
                                                          STATW + col + 1])

            def allreduce_cols(loc, tag):
                cin = dram.tile([C, 2], dt.float32, name=f"ccin_{tag}")
                cout = dram.tile([C, 2], dt.float32, addr_space="Shared",
                                 name=f"ccout_{tag}")
                nc.sync.dma_start(cin[:], loc[:])
                nc.gpsimd.collective_compute(
                    "AllReduce", Alu.add,
                    replica_groups=[list(range(N_CORES))],
                    ins=[cin.opt()], outs=[cout.opt()])
                glob = sb.tile([C, 2], dt.float32, name=f"glob_{tag}")
                nc.sync.dma_start(glob[:], cout[:])
                return glob

            def reduce_stats(stats_cols, tag, lo=0, hi=STATW):
                loc = sb.tile([C, 2], dt.float32, name=f"loc_{tag}")
                sc3 = stats_cols[:].rearrange("p (s i) -> p s i", s=2)
                nc.vector.tensor_reduce(loc[:], sc3[:, :, lo:hi],
                                        axis=mybir.AxisListType.X, op=Alu.add)
                return loc

            def bn_scalars(glob, gam, bet, tag):
                def t1(name):
                    return sb.tile([C, 1], dt.float32, name=f"{name}_{tag}")
                mean, e2, msq, var, ve = (t1("mean"), t1("e2"), t1("msq"),
                                          t1("var"), t1("ve"))
                nc.vector.tensor_scalar(mean[:], glob[:, 0:1], 1.0 / NTOT, None,
                                        Alu.mult)
                nc.vector.tensor_scalar(e2[:], glob[:, 1:2], 1.0 / NTOT, None,
                                        Alu.mult)
                nc.vector.tensor_tensor(msq[:], mean[:], mean[:], Alu.mult)
                nc.vector.tensor_tensor(var[:], e2[:], msq[:], Alu.subtract)
                nc.vector.tensor_scalar(ve[:], var[:], EPS, None, Alu.add)
                sq, y = t1("sq"), t1("y0")
                nc.scalar.activation(sq[:], ve[:], Act.Sqrt, bias=0.0, scale=1.0)
                nc.vector.reciprocal(y[:], sq[:])
                for it in range(2):  # Newton: y <- y*(1.5 - 0.5*ve*y^2)
                    tt1, tt2, tt3, yn = (t1(f"n{it}a"), t1(f"n{it}b"),
                                         t1(f"n{it}c"), t1(f"y{it + 1}"))
                    nc.vector.tensor_tensor(tt1[:], y[:], y[:], Alu.mult)
                    nc.vector.tensor_tensor(tt2[:], tt1[:], ve[:], Alu.mult)
                    nc.vector.tensor_scalar(tt3[:], tt2[:], -0.5, 1.5, Alu.mult,
                                            Alu.add)
                    nc.vector.tensor_tensor(yn[:], y[:], tt3[:], Alu.mult)
                    y = yn
                sbn, bt, sq_, bq_ = (t1("sbn"), t1("bt"), t1("sclq"), t1("biasq"))
                nc.vector.tensor_tensor(sbn[:], gam[:], y[:], Alu.mult)
                nc.vector.tensor_tensor(bt[:], mean[:], sbn[:], Alu.mult)
                # u = 255*(s*x + b): sq = 255*s ; bq = 255*(beta - mean*s)
                nc.vector.tensor_scalar(bq_[:], bt[:], -255.0, bet255(bet, tag),
                                        Alu.mult, Alu.add)
                nc.vector.tensor_scalar(sq_[:], sbn[:], 255.0, None, Alu.mult)
                return sq_, bq_

            _bet255 = {}

            def bet255(bet, tag):
                if tag not in _bet255:
                    b = sb.tile([C, 1], dt.float32, name=f"bet255_{tag}")
                    nc.vector.tensor_scalar(b[:], bet[:], 255.0, None, Alu.mult)
                    _bet255[tag] = b
                return _bet255[tag][:, 0:1]

            # ---------- quantize rows [r0, r1) of one image into the 8-bit grid ----------
            def quantize_img(src3d_full, A8t, sq_, bq_, i, tag, r0=0, r1=H):
                # u = relu(255*(s*x+b)); round+clamp: (min(u,255)+M)-M
                src3d = src3d_full[:, r0:r1]
                n = (r1 - r0) * W
                u = sb.tile([C, NPIX_IMG], dt.float32, tag="scr", bufs=2,
                            name=f"qu_{tag}_{i}_{r0}")
                u3 = u[:, 0:n].rearrange("p (a b) -> p a b", a=r1 - r0)
                nc.scalar.activation(u3, src3d, Act.Relu, bias=bq_[:, 0:1],
                                     scale=sq_[:, 0:1])
                nc.vector.tensor_scalar(u[:, 0:n], u[:, 0:n], 255.0, MAGIC,
                                        Alu.min, Alu.add)
                dst = A8t[:, i * IMG + BASE + r0 * WP:
                          i * IMG + BASE + r1 * WP]
                dst3 = dst.rearrange("p (a b) -> p a b", b=WP)[:, :, 0:W]
                nc.vector.tensor_scalar(dst3, u3, MAGIC, None, Alu.subtract)

            # ---------- weight preparation ----------
            def prep_amax(tag):
                # partition-axis max via PE transpose + DVE reduce + tiny DMA
                amax = sb.tile([C, 9], dt.float32, name=f"amax_{tag}")
                for k in KORDER:
                    nc.vector.tensor_reduce(amax[:, k:k + 1],
                                            raw9[:, k * 1152:(k + 1) * 1152],
                                            axis=mybir.AxisListType.X, op=Alu.max,
                                            apply_absolute_value=True)
                tp9 = ps.tile([9, 128], dt.float32, tag="tps", bufs=2,
                              name=f"tp9_{tag}")
                nc.tensor.transpose(tp9[:], amax[:], ident[:])
                mx9 = sb.tile([9, 1], dt.float32, name=f"mx9_{tag}")
                nc.vector.tensor_reduce(mx9[:], tp9[:], axis=mybir.AxisListType.X,
                                        op=Alu.max)
                mrow = sb.tile([1, 9], dt.float32, name=f"mrow_{tag}")
                for k in range(9):
                    nc.sync.dma_start(mrow[0:1, k:k + 1], mx9[k:k + 1, 0:1])
                tam = sb.tile([1, 9], dt.float32, name=f"tam_{tag}")
                nc.scalar.activation(tam[:], mrow[:], Act.Tanh, bias=0.0, scale=1.0)
                a2 = sb.tile([1, 9], dt.float32, name=f"a2_{tag}")
                nc.vector.tensor_scalar(a2[:], tam[:], 2.0, None, Alu.mult)
                r2r = sb.tile([1, 9], dt.float32, name=f"r2r_{tag}")
                nc.vector.reciprocal(r2r[:], a2[:])
                r2 = sb.tile([C, 9], dt.float32, name=f"r2_{tag}")
                nc.gpsimd.partition_broadcast(r2[:], r2r[:])
                return r2

            def prep_tanh(tag):
                # tanh in place over the resident raws (after amax extraction)
                for k in KORDER:
                    v = raw9[:, k * 1152:(k + 1) * 1152]
                    nc.scalar.activation(v, v, Act.Tanh, bias=0.0, scale=1.0)

            def prep_chain(r2, bcast, tag, g):
                """accumulate one ba-group's quantized candidates -> wacc."""
                wacc = None
                for pos, k in enumerate(GROUP_KS[g]):
                    th = raw9[:, k * 1152:(k + 1) * 1152]
                    # wn = th/(2amax)+0.5 ; u2 = wn*nw + M (rounds) ; m = u2-M
                    # (+0.5*nw must NOT fold into M: M+0.5nw isn't fp32-exact)
                    nc.vector.tensor_scalar(th, th, r2[:, k:k + 1], 0.5,
                                            Alu.mult, Alu.add)
                    nc.vector.tensor_scalar(th, th, float(NW[k]), MAGIC,
                                            Alu.mult, Alu.add)
                    nc.vector.tensor_scalar(th, th, MAGIC, None,
                                            Alu.subtract)
                    if pos == 0:
                        wacc = sb.tile([C, 1152], dt.float32, tag="wacc",
                                       bufs=2, name=f"wacc_{tag}_{g}_{pos}")
                        nc.vector.tensor_scalar(wacc[:], th, bcast[:, k:k + 1],
                                                bcast[:, 9 + g:10 + g],
                                                Alu.mult, Alu.add)
                    else:
                        nxt = sb.tile([C, 1152], dt.float32, tag="wacc",
                                      bufs=2, name=f"wacc_{tag}_{g}_{pos}")
                        nc.vector.scalar_tensor_tensor(nxt[:], th,
                                                       bcast[:, k:k + 1], wacc[:],
                                                       Alu.mult, Alu.add)
                        wacc = nxt
                return wacc

            def prep_transpose(wacc, tag, g, Wt):
                # PSUM->SBUF copies ride DVE (idle during prep) so the ACT
                # queue never stalls them behind the BN-gated quantize relu
                w3 = wacc[:].rearrange("p (i t) -> p i t", t=9)
                for t in range(9):
                    tp = ps.tile([128, 128], dt.float32, tag="tps", bufs=2,
                                 name=f"tp_{tag}_{g}_{t}")
                    nc.tensor.transpose(tp[:], w3[:, :, t], ident[:])
                    nc.vector.tensor_scalar(Wt[g][:, t, :], tp[:], WSCALE,
                                            None, Alu.mult)

            def alloc_W(tag):
                # fp16 x256: single-pass weights at 16-bit PE streaming rate
                return [sb.tile([C, 9, C], dt.float16, name=f"W_{tag}_{g}")
                        for g in range(3)]

            # ---------- conv pieces ----------
            def derive_sc(src8, s, tag):
                start = VSTART + s * SLEN
                lo = min(max((start - 64) & ~1, 0), APIX - STG)
                a4s = sb.tile([C, STG], dt.float16, tag="a4s", bufs=3,
                              name=f"a4_{tag}_{s}")
                nc.vector.tensor_scalar(a4s[:], src8[:, lo:lo + STG], 1.0 / 17.0,
                                        MAGICH, Alu.mult, Alu.add)
                nc.vector.tensor_scalar(a4s[:], a4s[:], MAGICH, None, Alu.subtract)
                a2s = sb.tile([C, STG], dt.float16, tag="a2s", bufs=3,
                              name=f"a2_{tag}_{s}")
                nc.vector.tensor_scalar(a2s[:], a4s[:], 1.0 / 5.0, MAGICH,
                                        Alu.mult, Alu.add)
                nc.vector.tensor_scalar(a2s[:], a2s[:], MAGICH, None, Alu.subtract)
                return a4s, a2s, lo

            def conv_sc(Wt, src8, a4s, a2s, lo, cdst, s, tag, groups=None,
                        tail_cb=None):
                """emit conv passes for superchunk s; groups=None -> all.

                tail_cb: chunk-major final pass; after each chunk's PSUM
                closes it is copied out and tail_cb(ci) emitted, so the
                critical path after the very last matmul is one tiny chunk.
                """
                chunks = _chunks_of_sc(s)
                start = VSTART + s * SLEN
                all_passes = [2, 1, 0]
                passes = [(pi, g) for pi, g in enumerate(all_passes)
                          if groups is None or g in groups]
                pt = _sc_psum(tag, s)

                def mm(g, t, pcol, gs, ln, pi):
                    off = TAPS[t]
                    if g == 2:
                        rhs = src8[:, gs + off:gs + off + ln]
                    elif g == 1:
                        rhs = a4s[:, gs + off - lo:gs + off - lo + ln]
                    else:
                        rhs = a2s[:, gs + off - lo:gs + off - lo + ln]
                    nc.tensor.matmul(
                        pt[:, pcol:pcol + ln], Wt[g][:, t, :], rhs,
                        start=(pi == 0 and t == 0),
                        stop=(pi == len(all_passes) - 1 and t == 8))

                for pi, g in passes:
                    if tail_cb is not None and pi == len(all_passes) - 1:
                        for ci, (pcol, gs, ln) in enumerate(chunks):
                            for t in range(9):
                                mm(g, t, pcol, gs, ln, pi)
                            nc.scalar.activation(cdst[:, gs:gs + ln],
                                                 pt[:, pcol:pcol + ln], Act.Copy,
                                                 bias=0.0, scale=1.0 / WSCALE)
                            tail_cb(ci)
                        return
                    for t in range(9):
                        for ci, (pcol, gs, ln) in enumerate(chunks):
                            mm(g, t, pcol, gs, ln, pi)
                if groups is None or all_passes[-1] in groups:
                    sc_end = min(start + SLEN, VEND)
                    nc.scalar.activation(cdst[:, start:sc_end],
                                         pt[:, 0:sc_end - start], Act.Copy,
                                         bias=0.0, scale=1.0 / WSCALE)

            _psums = {}

            def _sc_psum(tag, s):
                key = (tag, s)
                if key not in _psums:
                    _psums[key] = ps.tile([128, SLEN], dt.float32, tag="cps",
                                          bufs=2, name=f"ps_{tag}_{s}")
                return _psums[key]

            # ================= LAYER 1 =================
            stats1 = sb.tile([C, 2 * STATW], dt.float32)
            nc.vector.memset(stats1[:], 0.0)
            x3 = x_sb[:].rearrange("p (b a w) -> p b a w", b=BS, a=H)
            for i in range(BS):
                img_stats(x3[:, i], stats1, i, "s1",
                          src2d=x_sb[:, i * NPIX_IMG:(i + 1) * NPIX_IMG])

            loc1 = reduce_stats(stats1, "c1")
            glob1 = allreduce_cols(loc1, "c1")

            # DVE memset: keeps the gpsimd queue clear so the preamble
            # barrier collective triggers immediately
            nc.vector.memset(A8[:], 0.0)

            bc1 = softmax_strip(p1r, gn1r, "l1")
            bc2 = softmax_strip(p2r, gn2r, "l2")

            r2_1 = prep_amax("w1")
            prep_tanh("w1")

            W1 = alloc_W("w1")
            c1 = sb.tile([C, APIX], dt.float32, tag="big", name="c1buf")
            stats2 = sb.tile([C, 2 * STATW], dt.float32)
            nc.vector.memset(stats2[:], 0.0)

            # all weight prep happens before the BN-gated quantize so the
            # PE transposes' ACT copies never queue behind the glob1 wait
            for g in (2, 1, 0):
                wacc = prep_chain(r2_1, bc1, "w1", g)
                prep_transpose(wacc, "w1", g, W1)
            nsq1, nbq1 = bn_scalars(glob1, gam1, bet1, "bn1")
            quantize_img(x3[:, 0], A8, nsq1, nbq1, 0, "q1", 0, ROWS_SC0)
            a4s0, a2s0, lo0 = derive_sc(A8[:], 0, "cv1")
            quantize_img(x3[:, 0], A8, nsq1, nbq1, 0, "q1", ROWS_SC0, H)
            quantize_img(x3[:, 1], A8, nsq1, nbq1, 1, "q1")
            quantize_img(x3[:, 2], A8, nsq1, nbq1, 2, "q1")
            quantize_img(x3[:, 3], A8, nsq1, nbq1, 3, "q1")
            # NOTE: all of x must be consumed (quantized) before conv1's first
            # PSUM copy writes c1 -- they share one SBUF slot and the slot
            # handover is tile-granular.

            def c1_img3d(i, r0=0, r1=H):
                off = i * IMG + BASE + r0 * WP
                v = c1[:, off:off + (r1 - r0) * WP]
                return v.rearrange("p (a b) -> p a b", b=WP)[:, :, 0:W]

            conv_sc(W1, A8[:], a4s0, a2s0, lo0, c1, 0, "cv1")
            a4s1, a2s1, lo1 = derive_sc(A8[:], 1, "cv1")
            conv_sc(W1, A8[:], a4s1, a2s1, lo1, c1, 1, "cv1")

            def cv1_after_sc(s):
                if s == 7:  # partial img3 stats (rows 0..37 available)
                    img_stats(c1_img3d(3, 0, 37), stats2, 3, "s2")
                    loc2a = reduce_stats(stats2, "c2a", 0, 4)
                    return allreduce_cols(loc2a, "c2a")
                if s in IMG_LAST_SC:
                    i = IMG_LAST_SC[s]
                    if i != 3:
                        img_stats(c1_img3d(i), stats2, i, "s2")
                return None

            cv1_after_sc(0), cv1_after_sc(1)
            glob2a = None
            w2src = w2_in.ap().rearrange("k o i a b -> k o (i a b)")

            def cv1_tail(ci):
                r0, r1 = TAIL_PIECES[ci]
                img_stats(c1_img3d(3, r0, r1), stats2, 4 + ci, f"s2t{ci}")

            for s in range(2, NSC):
                a4s, a2s, lo = derive_sc(A8[:], s, "cv1")
                conv_sc(W1, A8[:], a4s, a2s, lo, c1, s, "cv1",
                        tail_cb=cv1_tail if s == NSC - 1 else None)
                ret = cv1_after_sc(s)
                if ret is not None:
                    glob2a = ret
                if s == 2:
                    # layer-2 raws reuse the raw9 slot once layer-1 chains
                    # are consumed; spread across conv1 superchunks
                    for k in KORDER:
                        for hh in range(2):
                            nc.sync.dma_start(
                                raw9[:, k * 1152 + hh * 576:
                                     k * 1152 + (hh + 1) * 576],
                                w2src[k][:, hh * 576:(hh + 1) * 576])
                elif s == 3:
                    r2_2 = prep_amax("w2")
                    prep_tanh("w2")
                    W2 = alloc_W("w2")
                    wacc_g = prep_chain(r2_2, bc2, "w2", 2)
                    prep_transpose(wacc_g, "w2", 2, W2)
                elif s == 4:
                    wacc_g = prep_chain(r2_2, bc2, "w2", 1)
                    prep_transpose(wacc_g, "w2", 1, W2)
                elif s == 5:
                    wacc_g = prep_chain(r2_2, bc2, "w2", 0)
                    prep_transpose(wacc_g, "w2", 0, W2)

            # ================= LAYER 2 =================
            # tail stats went out per-chunk during superchunk 8; only the
            # tiny allreduce of the three tail pieces rides the critical path
            loc2b = reduce_stats(stats2, "c2b", 4, 7)
            glob2b = allreduce_cols(loc2b, "c2b")
            glob2 = sb.tile([C, 2], dt.float32, name="glob2")
            nc.vector.tensor_tensor(glob2[:], glob2a[:], glob2b[:], Alu.add)
            nsq2, nbq2 = bn_scalars(glob2, gam2, bet2, "bn2")

            # layer 2 reuses the A8 grid tile (conv1 is done with it)
            out_v = out_dram.ap().rearrange("b c h w -> c b (h w)")

            def residual_out(i, r0=0, r1=H):
                n = (r1 - r0) * W
                xr = sb.tile([C, NPIX_IMG], dt.float32, tag="scr", bufs=2,
                             name=f"xr_{i}_{r0}")
                nc.sync.dma_start(xr[:, 0:n], x_src[:, i, r0 * W:r1 * W])
                xr3 = xr[:, 0:n].rearrange("p (a b) -> p a b", a=r1 - r0)
                nc.vector.tensor_tensor(xr3, xr3, c1_img3d(i, r0, r1), Alu.add)
                nc.sync.dma_start(out_v[:, i, r0 * W:r1 * W], xr[:, 0:n])

            quantize_img(c1_img3d(0), A8, nsq2, nbq2, 0, "q2", 0, ROWS_SC0)
            _q2_done = {}
            for s in range(0, NSC):
                need = SC_NEEDS_IMG[s]
                for i in range(BS):
                    if need >= i and not _q2_done.get(i):
                        if i == 0:
                            quantize_img(c1_img3d(0), A8, nsq2, nbq2, 0, "q2",
                                         ROWS_SC0, H)
                        else:
                            quantize_img(c1_img3d(i), A8, nsq2, nbq2, i, "q2")
                        _q2_done[i] = True
                a4s, a2s, lo = derive_sc(A8[:], s, "cv2")
                conv_sc(W2, A8[:], a4s, a2s, lo, c1, s, "cv2",
                        tail_cb=(lambda ci: residual_out(3, *TAIL_PIECES[ci]))
                        if s == NSC - 1 else None)
                if s == 7:
                    residual_out(3, 0, 37)
                if s in IMG_LAST_SC and IMG_LAST_SC[s] != 3:
                    residual_out(IMG_LAST_SC[s])

    nc.compile()

    if LDW_REUSE:
        # drop PE weight reloads that repeat the previous load verbatim (the
        # array already holds these weights); only sync-free loads come out
        for blk in nc.main_func.blocks:
            keep, prev = [], None
            for inst in blk.instructions:
                if isinstance(inst, mybir.InstLdweights):
                    w = inst.ins[0]
                    key = (w.memref, w.offset, str(w.ap))
                    si = inst.sync_info
                    clean = si is None or (len(si.on_wait) == 0
                                           and len(si.on_update) == 0)
                    if key == prev and clean:
                        continue
                    prev = key
                elif isinstance(inst, mybir.InstMatmult):
                    wdt = inst.ins[1].dtype
                    if inst.is_transpose or wdt == mybir.dt.float32:
                        prev = None  # self-loading matmul clobbers the array
                keep.append(inst)
            blk.instructions = keep
    return nc


def _consts():
    c = np.zeros((1, 27), np.float32)
    for k in range(9):
        c[0, k] = 2.0 / (NW[k] * NA[k])
        c[0, 9 + k] = 1.0 / NA[k]
        c[0, 18 + k] = float(NW[k])
    return c


def _in_maps(inputs):
    x = np.ascontiguousarray(inputs["x"], dtype=np.float32)
    shared = {
        "conv1_w": np.ascontiguousarray(inputs["conv1_w"], dtype=np.float32),
        "conv2_w": np.ascontiguousarray(inputs["conv2_w"], dtype=np.float32),
        "gamma1": np.ascontiguousarray(inputs["gamma1"], dtype=np.float32),
        "beta1": np.ascontiguousarray(inputs["beta1"], dtype=np.float32),
        "gamma2": np.ascontiguousarray(inputs["gamma2"], dtype=np.float32),
        "beta2": np.ascontiguousarray(inputs["beta2"], dtype=np.float32),
        "p1": np.ascontiguousarray(inputs["p1"], dtype=np.float32),
        "p2": np.ascontiguousarray(inputs["p2"], dtype=np.float32),
        "gn1": np.ascontiguousarray(inputs["gn1"], dtype=np.float32),
        "gn2": np.ascontiguousarray(inputs["gn2"], dtype=np.float32),
        "tau": np.asarray(inputs["tau"], dtype=np.float32).reshape(1),
        "consts": _consts(),
    }
    return [dict(shared, x=x[c * BS:(c + 1) * BS]) for c in range(N_CORES)]


def _get_nc():
    if "nc" not in _CACHE:
        _CACHE["nc"] = _build()
    return _CACHE["nc"]


def _run(in_maps, trace=False):
    nc = _get_nc()
    return bass_utils.run_bass_kernel_spmd(
        nc, in_maps, core_ids=list(range(N_CORES)), trace=trace)


def kernel(**inputs) -> np.ndarray:
    res = _run(_in_maps(inputs))
    return np.concatenate([res.results[c]["out"] for c in range(N_CORES)], axis=0)
